# revision 2
# baseline (speedup 1.0000x reference)
"""Trainium2 Bass kernel for nn_NonLocalLayer (non-local attention block).

Data-parallel over batch: 32 samples -> 8 NeuronCores, 4 samples/core.
Per sample (all matmuls bf16 inputs, fp32 PSUM accumulation):
    theta = w_st @ st            (LAT=512, S=512)
    phi   = w_lt @ lt            (LAT=512, L=2048)
    gT    = (w_g @ lt)^T         (L=2048, LAT=512)   [computed transposed]
    scT   = phi^T @ theta        (L, S)              [scores transposed]
    E     = exp(scT / sqrt(LAT)) (no max-subtract; scores are O(1))
    D     = sum_L E              (1, S)
    U     = g @ E                (LAT, S)
    att   = U / D + b_g          (softmax-normalized attention output)
    LN over all (LAT, S), * ln_w + ln_b, relu
    out   = w_out @ y + b_out    (C=2048, S=512)
"""

import numpy as np
import ml_dtypes

import concourse.bacc as bacc
import concourse.mybir as mybir
import concourse.tile as tile
from concourse import bass_utils

N_CORES = 8
NSAMP = 4          # samples per core
C = 2048           # st/lt feature channels
LAT = 512          # latent channels
S = 512            # num st positions
L = 2048           # num lt positions
LN_EPS = 1e-5
P = 128
KT = C // P        # 16 contraction tiles
MT_LAT = LAT // P  # 4
MT_L = L // P      # 16
MT_C = C // P      # 16
NCHUNK = 4         # L chunks of 512
CHW = L // NCHUNK  # 512
INV_SQRT_LAT = 1.0 / float(np.sqrt(np.float32(LAT)))

BF = mybir.dt.bfloat16
F32 = mybir.dt.float32
AF = mybir.ActivationFunctionType
ALU = mybir.AluOpType


def build_nc():
    nc = bacc.Bacc("TRN2", target_bir_lowering=False, debug=False)

    st_d = nc.dram_tensor("st", (NSAMP, C, S), BF, kind="ExternalInput")
    lt_d = nc.dram_tensor("lt", (NSAMP, C, L), BF, kind="ExternalInput")
    w_stT_d = nc.dram_tensor("w_stT", (C, LAT), BF, kind="ExternalInput")
    w_ltT_d = nc.dram_tensor("w_ltT", (C, LAT), BF, kind="ExternalInput")
    w_gT_d = nc.dram_tensor("w_gT", (C, LAT), BF, kind="ExternalInput")
    w_outT_d = nc.dram_tensor("w_outT", (LAT, C), BF, kind="ExternalInput")
    b_st_d = nc.dram_tensor("b_st", (LAT,), F32, kind="ExternalInput")
    b_lt_d = nc.dram_tensor("b_lt", (LAT,), F32, kind="ExternalInput")
    b_g_d = nc.dram_tensor("b_g", (LAT,), F32, kind="ExternalInput")
    b_out_d = nc.dram_tensor("b_out", (C,), F32, kind="ExternalInput")
    ln_w_d = nc.dram_tensor("ln_w", (LAT, S), F32, kind="ExternalInput")
    ln_b_d = nc.dram_tensor("ln_b", (LAT, S), F32, kind="ExternalInput")
    out_d = nc.dram_tensor("out", (NSAMP, C, S), F32, kind="ExternalOutput")

    with tile.TileContext(nc) as tc:
        build_tile_kernel(
            tc, st_d, lt_d, w_stT_d, w_ltT_d, w_gT_d, w_outT_d,
            b_st_d, b_lt_d, b_g_d, b_out_d, ln_w_d, ln_b_d, out_d,
        )
    nc.finalize()
    return nc


def build_tile_kernel(tc, st_d, lt_d, w_stT_d, w_ltT_d, w_gT_d, w_outT_d,
                      b_st_d, b_lt_d, b_g_d, b_out_d, ln_w_d, ln_b_d, out_d):
    nc = tc.nc
    from contextlib import ExitStack
    ctx = ExitStack()
    consts = ctx.enter_context(tc.tile_pool(name="consts", bufs=1))
    st_p = ctx.enter_context(tc.tile_pool(name="st", bufs=1))
    th_p = ctx.enter_context(tc.tile_pool(name="theta", bufs=1))
    lt_p = ctx.enter_context(tc.tile_pool(name="lt", bufs=2))
    phi_p = ctx.enter_context(tc.tile_pool(name="phi", bufs=2))
    g_p = ctx.enter_context(tc.tile_pool(name="g", bufs=1))
    e_p = ctx.enter_context(tc.tile_pool(name="E", bufs=1))
    tmp_p = ctx.enter_context(tc.tile_pool(name="tmp", bufs=2))
    att_p = ctx.enter_context(tc.tile_pool(name="att", bufs=1))
    y_p = ctx.enter_context(tc.tile_pool(name="y", bufs=1))
    ost_p = ctx.enter_context(tc.tile_pool(name="ost", bufs=2))
    small_p = ctx.enter_context(tc.tile_pool(name="small", bufs=2))
    ps_w = ctx.enter_context(tc.tile_pool(name="ps_work", bufs=4, space="PSUM"))
    ps_u = ctx.enter_context(tc.tile_pool(name="ps_u", bufs=4, space="PSUM"))

    # ---- constants / weights (loaded once) ----
    w_stT = consts.tile([P, KT, LAT], BF)
    w_ltT = consts.tile([P, KT, LAT], BF)
    w_gT = consts.tile([P, KT, LAT], BF)
    w_outT = consts.tile([P, MT_LAT, C], BF)
    nc.sync.dma_start(w_stT[:], w_stT_d.ap().rearrange("(o p) m -> p o m", p=P))
    nc.sync.dma_start(w_ltT[:], w_ltT_d.ap().rearrange("(o p) m -> p o m", p=P))
    nc.sync.dma_start(w_gT[:], w_gT_d.ap().rearrange("(o p) m -> p o m", p=P))
    nc.sync.dma_start(w_outT[:], w_outT_d.ap().rearrange("(o p) m -> p o m", p=P))
    ln_w = consts.tile([P, MT_LAT, S], F32)
    ln_b = consts.tile([P, MT_LAT, S], F32)
    nc.sync.dma_start(ln_w[:], ln_w_d.ap().rearrange("(o p) s -> p o s", p=P))
    nc.sync.dma_start(ln_b[:], ln_b_d.ap().rearrange("(o p) s -> p o s", p=P))
    b_st = consts.tile([P, MT_LAT], F32)
    b_lt = consts.tile([P, MT_LAT], F32)
    b_g = consts.tile([P, MT_LAT], F32)
    b_out = consts.tile([P, MT_C], F32)
    nc.sync.dma_start(b_st[:], b_st_d.ap().rearrange("(o p) -> p o", p=P))
    nc.sync.dma_start(b_lt[:], b_lt_d.ap().rearrange("(o p) -> p o", p=P))
    nc.sync.dma_start(b_g[:], b_g_d.ap().rearrange("(o p) -> p o", p=P))
    nc.sync.dma_start(b_out[:], b_out_d.ap().rearrange("(o p) -> p o", p=P))
    ones_col = consts.tile([P, 1], F32)
    nc.vector.memset(ones_col[:], 1.0)
    eps_t = consts.tile([1, 1], F32)
    nc.vector.memset(eps_t[:], LN_EPS)

    # Per-sample state carried between emission stages
    state = {}

    def emit_theta(i):
        st_sb = st_p.tile([P, KT, S], BF, tag="st")
        nc.sync.dma_start(st_sb[:], st_d.ap()[i].rearrange("(o p) s -> p o s", p=P))
        theta = th_p.tile([P, MT_LAT, S], BF, tag="theta")
        with nc.named_scope(f"theta{i}"):
            for m in range(MT_LAT):
                pt = ps_w.tile([P, S], F32, tag="work")
                for k in range(KT):
                    nc.tensor.matmul(pt[:], w_stT[:, k, m * P:(m + 1) * P],
                                     st_sb[:, k, :], start=(k == 0), stop=(k == KT - 1))
                nc.scalar.activation(theta[:, m, :], pt[:], AF.Identity,
                                     bias=b_st[:, m:m + 1])
        state[i] = {"theta": theta}

    def emit_chunks(i):
        sti = state[i]
        theta = sti["theta"]
        g_sb = g_p.tile([P, MT_L, LAT], BF, tag="g")
        e_sb = e_p.tile([P, MT_L, S], BF, tag="E")
        dacc = tmp_p.tile([P, S], F32, tag="dacc")
        with nc.named_scope(f"chunks{i}"):
            for cix in range(NCHUNK):
                lt_sb = lt_p.tile([P, KT, CHW], BF, tag="lt")
                nc.sync.dma_start(
                    lt_sb[:],
                    lt_d.ap()[i, :, cix * CHW:(cix + 1) * CHW]
                    .rearrange("(o p) l -> p o l", p=P))
                # phi (LAT x CHW) for this chunk
                phi_sb = phi_p.tile([P, MT_LAT, CHW], BF, tag="phi")
                for m in range(MT_LAT):
                    pt = ps_w.tile([P, CHW], F32, tag="work")
                    for k in range(KT):
                        nc.tensor.matmul(pt[:], w_ltT[:, k, m * P:(m + 1) * P],
                                         lt_sb[:, k, :], start=(k == 0),
                                         stop=(k == KT - 1))
                    nc.vector.tensor_scalar(phi_sb[:, m, :], pt[:],
                                            b_lt[:, m:m + 1], None, ALU.add)
                # gT (CHW x LAT), 4 L-part tiles
                for j in range(MT_LAT):
                    lk = cix * MT_LAT + j
                    pt = ps_w.tile([P, LAT], F32, tag="work")
                    for k in range(KT):
                        nc.tensor.matmul(pt[:], lt_sb[:, k, j * P:(j + 1) * P],
                                         w_gT[:, k, :], start=(k == 0),
                                         stop=(k == KT - 1))
                    nc.vector.tensor_copy(g_sb[:, lk, :], pt[:])
                # scores^T (CHW x S) then E = exp(sc/sqrt(LAT))
                for j in range(MT_LAT):
                    lk = cix * MT_LAT + j
                    pt = ps_w.tile([P, S], F32, tag="work")
                    for m in range(MT_LAT):
                        nc.tensor.matmul(pt[:], phi_sb[:, m, j * P:(j + 1) * P],
                                         theta[:, m, :], start=(m == 0),
                                         stop=(m == MT_LAT - 1))
                    nc.scalar.activation(e_sb[:, lk, :], pt[:], AF.Exp,
                                         scale=INV_SQRT_LAT)
                    if lk == 0:
                        nc.vector.tensor_copy(dacc[:], e_sb[:, 0, :])
                    else:
                        nc.vector.tensor_tensor(dacc[:], dacc[:], e_sb[:, lk, :],
                                                ALU.add)
        sti.update(g=g_sb, E=e_sb, dacc=dacc)

    def emit_attn(i):
        sti = state[i]
        g_sb, e_sb, dacc = sti["g"], sti["E"], sti["dacc"]
        with nc.named_scope(f"attn{i}"):
            psu = []
            for m in range(MT_LAT):
                pu = ps_u.tile([P, S], F32, tag="u")
                for lk in range(MT_L):
                    nc.tensor.matmul(pu[:], g_sb[:, lk, m * P:(m + 1) * P],
                                     e_sb[:, lk, :], start=(lk == 0),
                                     stop=(lk == MT_L - 1))
                psu.append(pu)
            # D = column sums of dacc across partitions (1 x S), fp32 matmul
            pd = ps_w.tile([1, S], F32, tag="work")
            nc.tensor.matmul(pd[:], ones_col[:], dacc[:], start=True, stop=True)
            r_sb = small_p.tile([1, S], F32, tag="r")
            nc.vector.reciprocal(r_sb[:], pd[:])
            rb = tmp_p.tile([P, S], F32, tag="rb")
            nc.gpsimd.partition_broadcast(rb[:], r_sb[:])
            att = att_p.tile([P, MT_LAT, S], F32, tag="att")
            for m in range(MT_LAT):
                nc.vector.tensor_tensor(att[:, m, :], psu[m][:], rb[:], ALU.mult)
                nc.vector.tensor_scalar(att[:, m, :], att[:, m, :],
                                        b_g[:, m:m + 1], None, ALU.add)
        sti["att"] = att

    def emit_ln_out(i):
        sti = state[i]
        att = sti["att"]
        with nc.named_scope(f"ln{i}"):
            # per-partition stats over the 4*S free elems
            stats = small_p.tile([P, MT_LAT, nc.vector.BN_STATS_DIM], F32, tag="bns")
            for m in range(MT_LAT):
                nc.vector.bn_stats(stats[:, m, :], att[:, m, :])
            mv = small_p.tile([P, nc.vector.BN_AGGR_DIM], F32, tag="bnm")
            nc.vector.bn_aggr(mv[:], stats[:])
            # pack [mean_p, mean_p^2 + var_p] then reduce across partitions
            t2 = small_p.tile([P, 2], F32, tag="t2")
            nc.vector.tensor_copy(t2[:, 0:1], mv[:, 0:1])
            nc.vector.tensor_tensor(t2[:, 1:2], mv[:, 0:1], mv[:, 0:1], ALU.mult)
            nc.vector.tensor_tensor(t2[:, 1:2], t2[:, 1:2], mv[:, 1:2], ALU.add)
            psm = ps_w.tile([1, 2], F32, tag="work")
            nc.tensor.matmul(psm[:], ones_col[:], t2[:], start=True, stop=True)
            sg = small_p.tile([1, 4], F32, tag="sg")
            # sg[0]=mu, sg[1]=E[x^2]
            nc.scalar.mul(sg[:, 0:2], psm[:], 1.0 / P)
            # var = E[x^2] - mu^2 -> sg[2]
            nc.vector.tensor_tensor(sg[:, 2:3], sg[:, 0:1], sg[:, 0:1], ALU.mult)
            nc.vector.tensor_tensor(sg[:, 2:3], sg[:, 1:2], sg[:, 2:3], ALU.subtract)
            # rstd = 1/sqrt(var + eps) -> sg[3]
            nc.scalar.activation(sg[:, 3:4], sg[:, 2:3], AF.Sqrt, bias=eps_t[:])
            nc.vector.reciprocal(sg[:, 3:4], sg[:, 3:4])
            musd = small_p.tile([1, 2], F32, tag="musd")
            nc.vector.tensor_copy(musd[:, 0:1], sg[:, 0:1])
            nc.vector.tensor_copy(musd[:, 1:2], sg[:, 3:4])
            musd_b = small_p.tile([P, 2], F32, tag="musdb")
            nc.gpsimd.partition_broadcast(musd_b[:], musd[:])
            y_sb = y_p.tile([P, MT_LAT, S], BF, tag="y")
            for m in range(MT_LAT):
                t = tmp_p.tile([P, S], F32, tag="lnt")
                nc.vector.tensor_scalar(t[:], att[:, m, :], musd_b[:, 0:1],
                                        musd_b[:, 1:2], ALU.subtract, ALU.mult)
                nc.vector.tensor_tensor(t[:], t[:], ln_w[:, m, :], ALU.mult)
                nc.vector.tensor_tensor(t[:], t[:], ln_b[:, m, :], ALU.add)
                nc.scalar.activation(y_sb[:, m, :], t[:], AF.Relu)
        with nc.named_scope(f"out{i}"):
            out_ap = out_d.ap()[i].rearrange("(o p) s -> p o s", p=P)
            for mo in range(MT_C):
                pt = ps_w.tile([P, S], F32, tag="work")
                for k in range(MT_LAT):
                    nc.tensor.matmul(pt[:], w_outT[:, k, mo * P:(mo + 1) * P],
                                     y_sb[:, k, :], start=(k == 0),
                                     stop=(k == MT_LAT - 1))
                ot = ost_p.tile([P, S], F32, tag="ost")
                nc.scalar.activation(ot[:], pt[:], AF.Identity,
                                     bias=b_out[:, mo:mo + 1])
                nc.sync.dma_start(out_ap[:, mo, :], ot[:])

    # Software-pipelined emission: sample i's LN/final overlaps sample i+1's
    # theta/projection matmuls on the PE.
    for i in range(NSAMP):
        emit_theta(i)
        emit_chunks(i)
        emit_attn(i)
        if i > 0:
            emit_ln_out(i - 1)
    emit_ln_out(NSAMP - 1)
    ctx.close()


_NC_CACHE = None


def _get_nc():
    global _NC_CACHE
    if _NC_CACHE is None:
        _NC_CACHE = build_nc()
    return _NC_CACHE


def kernel(st_feat, lt_feat, w_st, b_st, w_lt, b_lt, w_g, b_g,
           ln_w, ln_b, w_out, b_out):
    n = st_feat.shape[0]
    assert n == N_CORES * NSAMP
    bf16 = ml_dtypes.bfloat16
    st = np.asarray(st_feat, dtype=np.float32).reshape(n, C, S).astype(bf16)
    lt = np.asarray(lt_feat, dtype=np.float32).reshape(n, C, L).astype(bf16)
    w_stT = np.ascontiguousarray(np.asarray(w_st, np.float32).T).astype(bf16)
    w_ltT = np.ascontiguousarray(np.asarray(w_lt, np.float32).T).astype(bf16)
    w_gT = np.ascontiguousarray(np.asarray(w_g, np.float32).T).astype(bf16)
    w_outT = np.ascontiguousarray(np.asarray(w_out, np.float32).T).astype(bf16)
    shared = {
        "w_stT": w_stT, "w_ltT": w_ltT, "w_gT": w_gT, "w_outT": w_outT,
        "b_st": np.asarray(b_st, np.float32), "b_lt": np.asarray(b_lt, np.float32),
        "b_g": np.asarray(b_g, np.float32), "b_out": np.asarray(b_out, np.float32),
        "ln_w": np.ascontiguousarray(np.asarray(ln_w, np.float32)),
        "ln_b": np.ascontiguousarray(np.asarray(ln_b, np.float32)),
    }
    in_maps = []
    for c in range(N_CORES):
        sl = slice(c * NSAMP, (c + 1) * NSAMP)
        in_maps.append({"st": np.ascontiguousarray(st[sl]),
                        "lt": np.ascontiguousarray(lt[sl]), **shared})
    nc = _get_nc()
    res = bass_utils.run_bass_kernel_spmd(nc, in_maps, core_ids=list(range(N_CORES)))
    out = np.concatenate([res.results[c]["out"] for c in range(N_CORES)], axis=0)
    return out.reshape(n, C, S, 1, 1).astype(np.float32)


# revision 4
# speedup vs baseline: 1.0124x; 1.0124x over previous
"""Trainium2 Bass kernel for nn_NonLocalLayer (non-local attention block).

Data-parallel over batch: 32 samples -> 8 NeuronCores, 4 samples/core.
Per sample (all matmuls bf16 inputs, fp32 PSUM accumulation):
    theta = w_st @ st            (LAT=512, S=512)
    phi   = w_lt @ lt            (LAT=512, L=2048)
    gT    = (w_g @ lt)^T         (L=2048, LAT=512)   [computed transposed]
    scT   = phi^T @ theta        (L, S)              [scores transposed]
    E     = exp(scT / sqrt(LAT)) (no max-subtract; scores are O(1))
    D     = sum_L E              (1, S)
    U     = g @ E                (LAT, S)
    att   = U / D + b_g          (softmax-normalized attention output)
    LN over all (LAT, S), * ln_w + ln_b, relu
    out   = w_out @ y + b_out    (C=2048, S=512)
"""

import numpy as np
import ml_dtypes

import concourse.bacc as bacc
import concourse.mybir as mybir
import concourse.tile as tile
from concourse import bass_utils

N_CORES = 8
NSAMP = 4          # samples per core
C = 2048           # st/lt feature channels
LAT = 512          # latent channels
S = 512            # num st positions
L = 2048           # num lt positions
LN_EPS = 1e-5
P = 128
KT = C // P        # 16 contraction tiles
MT_LAT = LAT // P  # 4
MT_L = L // P      # 16
MT_C = C // P      # 16
NCHUNK = 4         # L chunks of 512
CHW = L // NCHUNK  # 512
INV_SQRT_LAT = 1.0 / float(np.sqrt(np.float32(LAT)))

BF = mybir.dt.bfloat16
F32 = mybir.dt.float32
AF = mybir.ActivationFunctionType
ALU = mybir.AluOpType


def build_nc():
    nc = bacc.Bacc("TRN2", target_bir_lowering=False, debug=False)

    st_d = nc.dram_tensor("st", (NSAMP, C, S), BF, kind="ExternalInput")
    lt_d = nc.dram_tensor("lt", (NSAMP, C, L), BF, kind="ExternalInput")
    w_stT_d = nc.dram_tensor("w_stT", (C, LAT), BF, kind="ExternalInput")
    w_ltT_d = nc.dram_tensor("w_ltT", (C, LAT), BF, kind="ExternalInput")
    w_gT_d = nc.dram_tensor("w_gT", (C, LAT), BF, kind="ExternalInput")
    w_outT_d = nc.dram_tensor("w_outT", (LAT, C), BF, kind="ExternalInput")
    b_st_d = nc.dram_tensor("b_st", (LAT,), F32, kind="ExternalInput")
    b_lt_d = nc.dram_tensor("b_lt", (LAT,), F32, kind="ExternalInput")
    b_g_d = nc.dram_tensor("b_g", (LAT,), F32, kind="ExternalInput")
    b_out_d = nc.dram_tensor("b_out", (C,), F32, kind="ExternalInput")
    ln_w_d = nc.dram_tensor("ln_w", (LAT, S), F32, kind="ExternalInput")
    ln_b_d = nc.dram_tensor("ln_b", (LAT, S), F32, kind="ExternalInput")
    out_d = nc.dram_tensor("out", (NSAMP, C, S), F32, kind="ExternalOutput")

    with tile.TileContext(nc) as tc:
        build_tile_kernel(
            tc, st_d, lt_d, w_stT_d, w_ltT_d, w_gT_d, w_outT_d,
            b_st_d, b_lt_d, b_g_d, b_out_d, ln_w_d, ln_b_d, out_d,
        )
    nc.finalize()
    return nc


def build_tile_kernel(tc, st_d, lt_d, w_stT_d, w_ltT_d, w_gT_d, w_outT_d,
                      b_st_d, b_lt_d, b_g_d, b_out_d, ln_w_d, ln_b_d, out_d):
    nc = tc.nc
    from contextlib import ExitStack
    ctx = ExitStack()
    consts = ctx.enter_context(tc.tile_pool(name="consts", bufs=1))
    st_p = ctx.enter_context(tc.tile_pool(name="st", bufs=1))
    th_p = ctx.enter_context(tc.tile_pool(name="theta", bufs=1))
    lt_p = ctx.enter_context(tc.tile_pool(name="lt", bufs=2))
    phi_p = ctx.enter_context(tc.tile_pool(name="phi", bufs=2))
    g_p = ctx.enter_context(tc.tile_pool(name="g", bufs=1))
    e_p = ctx.enter_context(tc.tile_pool(name="E", bufs=1))
    tmp_p = ctx.enter_context(tc.tile_pool(name="tmp", bufs=2))
    att_p = ctx.enter_context(tc.tile_pool(name="att", bufs=1))
    y_p = ctx.enter_context(tc.tile_pool(name="y", bufs=1))
    ost_p = ctx.enter_context(tc.tile_pool(name="ost", bufs=2))
    small_p = ctx.enter_context(tc.tile_pool(name="small", bufs=2))
    ps_w = ctx.enter_context(tc.tile_pool(name="ps_work", bufs=4, space="PSUM"))
    ps_u = ctx.enter_context(tc.tile_pool(name="ps_u", bufs=4, space="PSUM"))

    # ---- constants / weights (loaded once); ordered so the critical-path
    # weights (w_stT for theta, then w_ltT/w_gT for the chunk loop) arrive
    # first and the LN/output-stage constants load in the background.
    w_stT = consts.tile([P, KT, LAT], BF)
    w_ltT = consts.tile([P, KT, LAT], BF)
    w_gT = consts.tile([P, KT, LAT], BF)
    w_outT = consts.tile([P, MT_LAT, C], BF)
    b_st = consts.tile([P, MT_LAT], F32)
    b_lt = consts.tile([P, MT_LAT], F32)
    b_g = consts.tile([P, MT_LAT], F32)
    b_out = consts.tile([P, MT_C], F32)
    ln_w = consts.tile([P, MT_LAT, S], F32)
    ln_b = consts.tile([P, MT_LAT, S], F32)
    ones_col = consts.tile([P, 1], F32)
    eps_t = consts.tile([1, 1], F32)
    nc.sync.dma_start(w_stT[:], w_stT_d.ap().rearrange("(o p) m -> p o m", p=P))
    nc.sync.dma_start(b_st[:], b_st_d.ap().rearrange("(o p) -> p o", p=P))
    nc.sync.dma_start(w_ltT[:], w_ltT_d.ap().rearrange("(o p) m -> p o m", p=P))
    nc.sync.dma_start(w_gT[:], w_gT_d.ap().rearrange("(o p) m -> p o m", p=P))
    nc.sync.dma_start(b_lt[:], b_lt_d.ap().rearrange("(o p) -> p o", p=P))
    nc.sync.dma_start(b_g[:], b_g_d.ap().rearrange("(o p) -> p o", p=P))
    nc.vector.memset(ones_col[:], 1.0)
    nc.vector.memset(eps_t[:], LN_EPS)

    def load_late_consts():
        nc.sync.dma_start(w_outT[:], w_outT_d.ap().rearrange("(o p) m -> p o m", p=P))
        nc.sync.dma_start(ln_w[:], ln_w_d.ap().rearrange("(o p) s -> p o s", p=P))
        nc.sync.dma_start(ln_b[:], ln_b_d.ap().rearrange("(o p) s -> p o s", p=P))
        nc.sync.dma_start(b_out[:], b_out_d.ap().rearrange("(o p) -> p o", p=P))

    # Per-sample state carried between emission stages
    state = {}

    def emit_theta(i):
        st_sb = st_p.tile([P, KT, S], BF, tag="st")
        nc.sync.dma_start(st_sb[:], st_d.ap()[i].rearrange("(o p) s -> p o s", p=P))
        theta = th_p.tile([P, MT_LAT, S], BF, tag="theta")
        with nc.named_scope(f"theta{i}"):
            for m in range(MT_LAT):
                pt = ps_w.tile([P, S], F32, tag="work")
                for k in range(KT):
                    nc.tensor.matmul(pt[:], w_stT[:, k, m * P:(m + 1) * P],
                                     st_sb[:, k, :], start=(k == 0), stop=(k == KT - 1))
                nc.scalar.activation(theta[:, m, :], pt[:], AF.Identity,
                                     bias=b_st[:, m:m + 1])
        state[i] = {"theta": theta}

    def emit_chunks(i):
        sti = state[i]
        theta = sti["theta"]
        g_sb = g_p.tile([P, MT_L, LAT], BF, tag="g")
        e_sb = e_p.tile([P, MT_L, S], BF, tag="E")
        dacc = tmp_p.tile([P, S], F32, tag="dacc")
        with nc.named_scope(f"chunks{i}"):
            for cix in range(NCHUNK):
                lt_sb = lt_p.tile([P, KT, CHW], BF, tag="lt")
                nc.sync.dma_start(
                    lt_sb[:],
                    lt_d.ap()[i, :, cix * CHW:(cix + 1) * CHW]
                    .rearrange("(o p) l -> p o l", p=P))
                # phi (LAT x CHW) for this chunk
                phi_sb = phi_p.tile([P, MT_LAT, CHW], BF, tag="phi")
                for m in range(MT_LAT):
                    pt = ps_w.tile([P, CHW], F32, tag="work")
                    for k in range(KT):
                        nc.tensor.matmul(pt[:], w_ltT[:, k, m * P:(m + 1) * P],
                                         lt_sb[:, k, :], start=(k == 0),
                                         stop=(k == KT - 1))
                    nc.vector.tensor_scalar(phi_sb[:, m, :], pt[:],
                                            b_lt[:, m:m + 1], None, ALU.add)
                # gT (CHW x LAT), 4 L-part tiles
                for j in range(MT_LAT):
                    lk = cix * MT_LAT + j
                    pt = ps_w.tile([P, LAT], F32, tag="work")
                    for k in range(KT):
                        nc.tensor.matmul(pt[:], lt_sb[:, k, j * P:(j + 1) * P],
                                         w_gT[:, k, :], start=(k == 0),
                                         stop=(k == KT - 1))
                    nc.vector.tensor_copy(g_sb[:, lk, :], pt[:])
                # scores^T (CHW x S) then E = exp(sc/sqrt(LAT))
                for j in range(MT_LAT):
                    lk = cix * MT_LAT + j
                    pt = ps_w.tile([P, S], F32, tag="work")
                    for m in range(MT_LAT):
                        nc.tensor.matmul(pt[:], phi_sb[:, m, j * P:(j + 1) * P],
                                         theta[:, m, :], start=(m == 0),
                                         stop=(m == MT_LAT - 1))
                    nc.scalar.activation(e_sb[:, lk, :], pt[:], AF.Exp,
                                         scale=INV_SQRT_LAT)
                    if lk == 0:
                        nc.vector.tensor_copy(dacc[:], e_sb[:, 0, :])
                    else:
                        nc.vector.tensor_tensor(dacc[:], dacc[:], e_sb[:, lk, :],
                                                ALU.add)
        sti.update(g=g_sb, E=e_sb, dacc=dacc)

    def emit_attn(i):
        sti = state[i]
        g_sb, e_sb, dacc = sti["g"], sti["E"], sti["dacc"]
        with nc.named_scope(f"attn{i}"):
            psu = []
            for m in range(MT_LAT):
                pu = ps_u.tile([P, S], F32, tag="u")
                for lk in range(MT_L):
                    nc.tensor.matmul(pu[:], g_sb[:, lk, m * P:(m + 1) * P],
                                     e_sb[:, lk, :], start=(lk == 0),
                                     stop=(lk == MT_L - 1))
                psu.append(pu)
            # D = column sums of dacc across partitions (1 x S), fp32 matmul
            pd = ps_w.tile([1, S], F32, tag="work")
            nc.tensor.matmul(pd[:], ones_col[:], dacc[:], start=True, stop=True)
            r_sb = small_p.tile([1, S], F32, tag="r")
            nc.vector.reciprocal(r_sb[:], pd[:])
            rb = tmp_p.tile([P, S], F32, tag="rb")
            nc.gpsimd.partition_broadcast(rb[:], r_sb[:])
            att = att_p.tile([P, MT_LAT, S], F32, tag="att")
            for m in range(MT_LAT):
                nc.vector.tensor_tensor(att[:, m, :], psu[m][:], rb[:], ALU.mult)
                nc.vector.tensor_scalar(att[:, m, :], att[:, m, :],
                                        b_g[:, m:m + 1], None, ALU.add)
        sti["att"] = att

    def emit_ln_out(i):
        sti = state[i]
        att = sti["att"]
        with nc.named_scope(f"ln{i}"):
            # per-partition stats over the 4*S free elems
            stats = small_p.tile([P, MT_LAT, nc.vector.BN_STATS_DIM], F32, tag="bns")
            for m in range(MT_LAT):
                nc.vector.bn_stats(stats[:, m, :], att[:, m, :])
            mv = small_p.tile([P, nc.vector.BN_AGGR_DIM], F32, tag="bnm")
            nc.vector.bn_aggr(mv[:], stats[:])
            # pack [mean_p, mean_p^2 + var_p] then reduce across partitions
            t2 = small_p.tile([P, 2], F32, tag="t2")
            nc.vector.tensor_copy(t2[:, 0:1], mv[:, 0:1])
            nc.vector.tensor_tensor(t2[:, 1:2], mv[:, 0:1], mv[:, 0:1], ALU.mult)
            nc.vector.tensor_tensor(t2[:, 1:2], t2[:, 1:2], mv[:, 1:2], ALU.add)
            psm = ps_w.tile([1, 2], F32, tag="work")
            nc.tensor.matmul(psm[:], ones_col[:], t2[:], start=True, stop=True)
            sg = small_p.tile([1, 4], F32, tag="sg")
            # sg[0]=mu, sg[1]=E[x^2]
            nc.scalar.mul(sg[:, 0:2], psm[:], 1.0 / P)
            # var = E[x^2] - mu^2 -> sg[2]
            nc.vector.tensor_tensor(sg[:, 2:3], sg[:, 0:1], sg[:, 0:1], ALU.mult)
            nc.vector.tensor_tensor(sg[:, 2:3], sg[:, 1:2], sg[:, 2:3], ALU.subtract)
            # rstd = 1/sqrt(var + eps) -> sg[3]
            nc.scalar.activation(sg[:, 3:4], sg[:, 2:3], AF.Sqrt, bias=eps_t[:])
            nc.vector.reciprocal(sg[:, 3:4], sg[:, 3:4])
            musd = small_p.tile([1, 2], F32, tag="musd")
            nc.vector.tensor_copy(musd[:, 0:1], sg[:, 0:1])
            nc.vector.tensor_copy(musd[:, 1:2], sg[:, 3:4])
            musd_b = small_p.tile([P, 2], F32, tag="musdb")
            nc.gpsimd.partition_broadcast(musd_b[:], musd[:])
            y_sb = y_p.tile([P, MT_LAT, S], BF, tag="y")
            for m in range(MT_LAT):
                t = tmp_p.tile([P, S], F32, tag="lnt")
                nc.vector.tensor_scalar(t[:], att[:, m, :], musd_b[:, 0:1],
                                        musd_b[:, 1:2], ALU.subtract, ALU.mult)
                nc.vector.tensor_tensor(t[:], t[:], ln_w[:, m, :], ALU.mult)
                nc.vector.tensor_tensor(t[:], t[:], ln_b[:, m, :], ALU.add)
                nc.scalar.activation(y_sb[:, m, :], t[:], AF.Relu)
        with nc.named_scope(f"out{i}"):
            out_ap = out_d.ap()[i].rearrange("(o p) s -> p o s", p=P)
            for mo in range(MT_C):
                pt = ps_w.tile([P, S], F32, tag="work")
                for k in range(MT_LAT):
                    nc.tensor.matmul(pt[:], w_outT[:, k, mo * P:(mo + 1) * P],
                                     y_sb[:, k, :], start=(k == 0),
                                     stop=(k == MT_LAT - 1))
                ot = ost_p.tile([P, S], F32, tag="ost")
                nc.scalar.activation(ot[:], pt[:], AF.Identity,
                                     bias=b_out[:, mo:mo + 1])
                nc.sync.dma_start(out_ap[:, mo, :], ot[:])

    # Software-pipelined emission: sample i's LN/final work is emitted right
    # after sample i+1's theta so its serial Vector chain overlaps theta/chunk
    # matmuls on the PE (and its final matmuls slot in before the chunks).
    for i in range(NSAMP):
        emit_theta(i)
        if i == 0:
            load_late_consts()
        if i > 0:
            emit_ln_out(i - 1)
        emit_chunks(i)
        emit_attn(i)
    emit_ln_out(NSAMP - 1)
    ctx.close()


_NC_CACHE = None


def _get_nc():
    global _NC_CACHE
    if _NC_CACHE is None:
        _NC_CACHE = build_nc()
    return _NC_CACHE


def kernel(st_feat, lt_feat, w_st, b_st, w_lt, b_lt, w_g, b_g,
           ln_w, ln_b, w_out, b_out):
    n = st_feat.shape[0]
    assert n == N_CORES * NSAMP
    bf16 = ml_dtypes.bfloat16
    st = np.asarray(st_feat, dtype=np.float32).reshape(n, C, S).astype(bf16)
    lt = np.asarray(lt_feat, dtype=np.float32).reshape(n, C, L).astype(bf16)
    w_stT = np.ascontiguousarray(np.asarray(w_st, np.float32).T).astype(bf16)
    w_ltT = np.ascontiguousarray(np.asarray(w_lt, np.float32).T).astype(bf16)
    w_gT = np.ascontiguousarray(np.asarray(w_g, np.float32).T).astype(bf16)
    w_outT = np.ascontiguousarray(np.asarray(w_out, np.float32).T).astype(bf16)
    shared = {
        "w_stT": w_stT, "w_ltT": w_ltT, "w_gT": w_gT, "w_outT": w_outT,
        "b_st": np.asarray(b_st, np.float32), "b_lt": np.asarray(b_lt, np.float32),
        "b_g": np.asarray(b_g, np.float32), "b_out": np.asarray(b_out, np.float32),
        "ln_w": np.ascontiguousarray(np.asarray(ln_w, np.float32)),
        "ln_b": np.ascontiguousarray(np.asarray(ln_b, np.float32)),
    }
    in_maps = []
    for c in range(N_CORES):
        sl = slice(c * NSAMP, (c + 1) * NSAMP)
        in_maps.append({"st": np.ascontiguousarray(st[sl]),
                        "lt": np.ascontiguousarray(lt[sl]), **shared})
    nc = _get_nc()
    res = bass_utils.run_bass_kernel_spmd(nc, in_maps, core_ids=list(range(N_CORES)))
    out = np.concatenate([res.results[c]["out"] for c in range(N_CORES)], axis=0)
    return out.reshape(n, C, S, 1, 1).astype(np.float32)


# revision 9
# speedup vs baseline: 1.0182x; 1.0057x over previous
"""Trainium2 Bass kernel for nn_NonLocalLayer (non-local attention block).

Data-parallel over batch: 32 samples -> 8 NeuronCores, 4 samples/core.
Per sample (all matmuls bf16 inputs, fp32 PSUM accumulation):
    theta = w_st @ st            (LAT=512, S=512)
    phi   = w_lt @ lt            (LAT=512, L=2048)
    gT    = (w_g @ lt)^T         (L=2048, LAT=512)   [computed transposed]
    scT   = phi^T @ theta        (L, S)              [scores transposed]
    E     = exp(scT / sqrt(LAT)) (no max-subtract; scores are O(1))
    D     = sum_L E              (1, S)
    U     = g @ E                (LAT, S)
    att   = U / D + b_g          (softmax-normalized attention output)
    LN over all (LAT, S), * ln_w + ln_b, relu
    out   = w_out @ y + b_out    (C=2048, S=512)
"""

import numpy as np
import ml_dtypes

import concourse.bacc as bacc
import concourse.mybir as mybir
import concourse.tile as tile
from concourse import bass_utils

N_CORES = 8
NSAMP = 4          # samples per core
C = 2048           # st/lt feature channels
LAT = 512          # latent channels
S = 512            # num st positions
L = 2048           # num lt positions
LN_EPS = 1e-5
P = 128
KT = C // P        # 16 contraction tiles
MT_LAT = LAT // P  # 4
MT_L = L // P      # 16
MT_C = C // P      # 16
NCHUNK = 4         # L chunks of 512
CHW = L // NCHUNK  # 512
INV_SQRT_LAT = 1.0 / float(np.sqrt(np.float32(LAT)))

BF = mybir.dt.bfloat16
F32 = mybir.dt.float32
AF = mybir.ActivationFunctionType
ALU = mybir.AluOpType


def build_nc():
    nc = bacc.Bacc("TRN2", target_bir_lowering=False, debug=False)

    st_d = nc.dram_tensor("st", (NSAMP, C, S), BF, kind="ExternalInput")
    lt_d = nc.dram_tensor("lt", (NSAMP, C, L), BF, kind="ExternalInput")
    w_stT_d = nc.dram_tensor("w_stT", (C, LAT), BF, kind="ExternalInput")
    w_ltT_d = nc.dram_tensor("w_ltT", (C, LAT), BF, kind="ExternalInput")
    w_gT_d = nc.dram_tensor("w_gT", (C, LAT), BF, kind="ExternalInput")
    w_outT_d = nc.dram_tensor("w_outT", (LAT, C), BF, kind="ExternalInput")
    b_st_d = nc.dram_tensor("b_st", (LAT,), F32, kind="ExternalInput")
    b_lt_d = nc.dram_tensor("b_lt", (LAT,), F32, kind="ExternalInput")
    b_g_d = nc.dram_tensor("b_g", (LAT,), F32, kind="ExternalInput")
    b_out_d = nc.dram_tensor("b_out", (C,), F32, kind="ExternalInput")
    ln_w_d = nc.dram_tensor("ln_w", (LAT, S), F32, kind="ExternalInput")
    ln_b_d = nc.dram_tensor("ln_b", (LAT, S), F32, kind="ExternalInput")
    out_d = nc.dram_tensor("out", (NSAMP, C, S), F32, kind="ExternalOutput")

    with tile.TileContext(nc) as tc:
        build_tile_kernel(
            tc, st_d, lt_d, w_stT_d, w_ltT_d, w_gT_d, w_outT_d,
            b_st_d, b_lt_d, b_g_d, b_out_d, ln_w_d, ln_b_d, out_d,
        )
    nc.finalize()
    return nc


def build_tile_kernel(tc, st_d, lt_d, w_stT_d, w_ltT_d, w_gT_d, w_outT_d,
                      b_st_d, b_lt_d, b_g_d, b_out_d, ln_w_d, ln_b_d, out_d):
    nc = tc.nc
    from contextlib import ExitStack
    ctx = ExitStack()
    consts = ctx.enter_context(tc.tile_pool(name="consts", bufs=1))
    st_p = ctx.enter_context(tc.tile_pool(name="st", bufs=1))
    th_p = ctx.enter_context(tc.tile_pool(name="theta", bufs=1))
    lt_p = ctx.enter_context(tc.tile_pool(name="lt", bufs=2))
    phi_p = ctx.enter_context(tc.tile_pool(name="phi", bufs=2))
    g_p = ctx.enter_context(tc.tile_pool(name="g", bufs=1))
    e_p = ctx.enter_context(tc.tile_pool(name="E", bufs=1))
    tmp_p = ctx.enter_context(tc.tile_pool(name="tmp", bufs=2))
    att_p = ctx.enter_context(tc.tile_pool(name="att", bufs=1))
    y_p = ctx.enter_context(tc.tile_pool(name="y", bufs=1))
    ost_p = ctx.enter_context(tc.tile_pool(name="ost", bufs=2))
    small_p = ctx.enter_context(tc.tile_pool(name="small", bufs=2))
    ps_w = ctx.enter_context(tc.tile_pool(name="ps_work", bufs=4, space="PSUM"))
    ps_u = ctx.enter_context(tc.tile_pool(name="ps_u", bufs=4, space="PSUM"))

    # ---- constants / weights (loaded once); ordered so the critical-path
    # weights (w_stT for theta, then w_ltT/w_gT for the chunk loop) arrive
    # first and the LN/output-stage constants load in the background.
    w_stT = consts.tile([P, KT, LAT], BF)
    w_ltT = consts.tile([P, KT, LAT], BF)
    w_gT = consts.tile([P, KT, LAT], BF)
    w_outT = consts.tile([P, MT_LAT, C], BF)
    b_st = consts.tile([P, MT_LAT], F32)
    b_lt = consts.tile([P, MT_LAT], F32)
    b_g = consts.tile([P, MT_LAT], F32)
    b_out = consts.tile([P, MT_C], F32)
    ln_w = consts.tile([P, MT_LAT, S], F32)
    ln_b = consts.tile([P, MT_LAT, S], F32)
    ones_col = consts.tile([P, 1], F32)
    eps_t = consts.tile([1, 1], F32)
    nc.sync.dma_start(w_stT[:], w_stT_d.ap().rearrange("(o p) m -> p o m", p=P))
    nc.sync.dma_start(b_st[:], b_st_d.ap().rearrange("(o p) -> p o", p=P))
    nc.sync.dma_start(w_ltT[:], w_ltT_d.ap().rearrange("(o p) m -> p o m", p=P))
    nc.sync.dma_start(w_gT[:], w_gT_d.ap().rearrange("(o p) m -> p o m", p=P))
    nc.sync.dma_start(b_lt[:], b_lt_d.ap().rearrange("(o p) -> p o", p=P))
    nc.sync.dma_start(b_g[:], b_g_d.ap().rearrange("(o p) -> p o", p=P))
    nc.vector.memset(ones_col[:], 1.0)
    nc.vector.memset(eps_t[:], LN_EPS)

    def load_late_consts():
        nc.sync.dma_start(w_outT[:], w_outT_d.ap().rearrange("(o p) m -> p o m", p=P))
        nc.sync.dma_start(ln_w[:], ln_w_d.ap().rearrange("(o p) s -> p o s", p=P))
        nc.sync.dma_start(ln_b[:], ln_b_d.ap().rearrange("(o p) s -> p o s", p=P))
        nc.sync.dma_start(b_out[:], b_out_d.ap().rearrange("(o p) -> p o", p=P))

    # Per-sample state carried between emission stages
    state = {}

    def emit_theta(i):
        st_sb = st_p.tile([P, KT, S], BF, tag="st")
        nc.sync.dma_start(st_sb[:], st_d.ap()[i].rearrange("(o p) s -> p o s", p=P))
        theta = th_p.tile([P, MT_LAT, S], BF, tag="theta")
        with nc.named_scope(f"theta{i}"):
            for m in range(MT_LAT):
                pt = ps_w.tile([P, S], F32, tag="work")
                for k in range(KT):
                    nc.tensor.matmul(pt[:], w_stT[:, k, m * P:(m + 1) * P],
                                     st_sb[:, k, :], start=(k == 0), stop=(k == KT - 1))
                nc.scalar.activation(theta[:, m, :], pt[:], AF.Identity,
                                     bias=b_st[:, m:m + 1])
        state[i] = {"theta": theta}

    def emit_chunk(i, cix):
        sti = state[i]
        theta = sti["theta"]
        if cix == 0:
            sti["g"] = g_p.tile([P, MT_L, LAT], BF, tag="g", name="g_sb")
            sti["E"] = e_p.tile([P, MT_L, S], BF, tag="E", name="e_sb")
            sti["dacc"] = tmp_p.tile([P, S], F32, tag="dacc", name="dacc")
        g_sb, e_sb, dacc = sti["g"], sti["E"], sti["dacc"]
        with nc.named_scope(f"chunks{i}"):
            if True:
                lt_sb = lt_p.tile([P, KT, CHW], BF, tag="lt")
                nc.sync.dma_start(
                    lt_sb[:],
                    lt_d.ap()[i, :, cix * CHW:(cix + 1) * CHW]
                    .rearrange("(o p) l -> p o l", p=P))
                # phi (LAT x CHW) for this chunk
                phi_sb = phi_p.tile([P, MT_LAT, CHW], BF, tag="phi")
                for m in range(MT_LAT):
                    pt = ps_w.tile([P, CHW], F32, tag="work")
                    for k in range(KT):
                        nc.tensor.matmul(pt[:], w_ltT[:, k, m * P:(m + 1) * P],
                                         lt_sb[:, k, :], start=(k == 0),
                                         stop=(k == KT - 1))
                    nc.vector.tensor_scalar(phi_sb[:, m, :], pt[:],
                                            b_lt[:, m:m + 1], None, ALU.add)
                # gT (CHW x LAT), 4 L-part tiles
                for j in range(MT_LAT):
                    lk = cix * MT_LAT + j
                    pt = ps_w.tile([P, LAT], F32, tag="work")
                    for k in range(KT):
                        nc.tensor.matmul(pt[:], lt_sb[:, k, j * P:(j + 1) * P],
                                         w_gT[:, k, :], start=(k == 0),
                                         stop=(k == KT - 1))
                    nc.vector.tensor_copy(g_sb[:, lk, :], pt[:])
                # scores^T (CHW x S) then E = exp(sc/sqrt(LAT))
                for j in range(MT_LAT):
                    lk = cix * MT_LAT + j
                    pt = ps_w.tile([P, S], F32, tag="work")
                    for m in range(MT_LAT):
                        nc.tensor.matmul(pt[:], phi_sb[:, m, j * P:(j + 1) * P],
                                         theta[:, m, :], start=(m == 0),
                                         stop=(m == MT_LAT - 1))
                    nc.scalar.activation(e_sb[:, lk, :], pt[:], AF.Exp,
                                         scale=INV_SQRT_LAT)
                    if lk == 0:
                        nc.vector.tensor_copy(dacc[:], e_sb[:, 0, :])
                    else:
                        nc.vector.tensor_tensor(dacc[:], dacc[:], e_sb[:, lk, :],
                                                ALU.add)

    def emit_attn(i):
        sti = state[i]
        g_sb, e_sb, dacc = sti["g"], sti["E"], sti["dacc"]
        with nc.named_scope(f"attn{i}"):
            psu = []
            for m in range(MT_LAT):
                pu = ps_u.tile([P, S], F32, tag="u")
                for lk in range(MT_L):
                    nc.tensor.matmul(pu[:], g_sb[:, lk, m * P:(m + 1) * P],
                                     e_sb[:, lk, :], start=(lk == 0),
                                     stop=(lk == MT_L - 1))
                psu.append(pu)
            # D = column sums of dacc across partitions (1 x S), fp32 matmul
            pd = ps_w.tile([1, S], F32, tag="work")
            nc.tensor.matmul(pd[:], ones_col[:], dacc[:], start=True, stop=True)
            r_sb = small_p.tile([1, S], F32, tag="r")
            nc.vector.reciprocal(r_sb[:], pd[:])
            rb = tmp_p.tile([P, S], F32, tag="rb")
            nc.gpsimd.partition_broadcast(rb[:], r_sb[:])
            att = att_p.tile([P, MT_LAT, S], F32, tag="att")
            for m in range(MT_LAT):
                nc.vector.tensor_tensor(att[:, m, :], psu[m][:], rb[:], ALU.mult)
                nc.vector.tensor_scalar(att[:, m, :], att[:, m, :],
                                        b_g[:, m:m + 1], None, ALU.add)
        sti["att"] = att

    def emit_ln_out(i):
        sti = state[i]
        att = sti["att"]
        with nc.named_scope(f"ln{i}"):
            # per-partition stats over the 4*S free elems
            stats = small_p.tile([P, MT_LAT, nc.vector.BN_STATS_DIM], F32, tag="bns")
            for m in range(MT_LAT):
                nc.vector.bn_stats(stats[:, m, :], att[:, m, :])
            mv = small_p.tile([P, nc.vector.BN_AGGR_DIM], F32, tag="bnm")
            nc.vector.bn_aggr(mv[:], stats[:])
            # pack [mean_p, mean_p^2 + var_p] then reduce across partitions
            t2 = small_p.tile([P, 2], F32, tag="t2")
            nc.vector.tensor_copy(t2[:, 0:1], mv[:, 0:1])
            nc.vector.tensor_tensor(t2[:, 1:2], mv[:, 0:1], mv[:, 0:1], ALU.mult)
            nc.vector.tensor_tensor(t2[:, 1:2], t2[:, 1:2], mv[:, 1:2], ALU.add)
            psm = ps_w.tile([1, 2], F32, tag="work")
            nc.tensor.matmul(psm[:], ones_col[:], t2[:], start=True, stop=True)
            sg = small_p.tile([1, 4], F32, tag="sg")
            # sg[0]=mu, sg[1]=E[x^2]
            nc.scalar.mul(sg[:, 0:2], psm[:], 1.0 / P)
            # var = E[x^2] - mu^2 -> sg[2]
            nc.vector.tensor_tensor(sg[:, 2:3], sg[:, 0:1], sg[:, 0:1], ALU.mult)
            nc.vector.tensor_tensor(sg[:, 2:3], sg[:, 1:2], sg[:, 2:3], ALU.subtract)
            # rstd = 1/sqrt(var + eps) -> sg[3]
            nc.scalar.activation(sg[:, 3:4], sg[:, 2:3], AF.Sqrt, bias=eps_t[:])
            nc.vector.reciprocal(sg[:, 3:4], sg[:, 3:4])
            musd = small_p.tile([1, 2], F32, tag="musd")
            nc.vector.tensor_copy(musd[:, 0:1], sg[:, 0:1])
            nc.vector.tensor_copy(musd[:, 1:2], sg[:, 3:4])
            musd_b = small_p.tile([P, 2], F32, tag="musdb")
            nc.gpsimd.partition_broadcast(musd_b[:], musd[:])
            y_sb = y_p.tile([P, MT_LAT, S], BF, tag="y")
            for m in range(MT_LAT):
                t = tmp_p.tile([P, S], F32, tag="lnt")
                nc.vector.tensor_scalar(t[:], att[:, m, :], musd_b[:, 0:1],
                                        musd_b[:, 1:2], ALU.subtract, ALU.mult)
                nc.vector.tensor_tensor(t[:], t[:], ln_w[:, m, :], ALU.mult)
                nc.vector.tensor_tensor(t[:], t[:], ln_b[:, m, :], ALU.add)
                nc.scalar.activation(y_sb[:, m, :], t[:], AF.Relu)
        with nc.named_scope(f"out{i}"):
            out_ap = out_d.ap()[i].rearrange("(o p) s -> p o s", p=P)
            for mo in range(MT_C):
                pool = ps_w if mo % 2 == 0 else ps_u
                pt = pool.tile([P, S], F32, tag="work" if mo % 2 == 0 else "u")
                for k in range(MT_LAT):
                    nc.tensor.matmul(pt[:], w_outT[:, k, mo * P:(mo + 1) * P],
                                     y_sb[:, k, :], start=(k == 0),
                                     stop=(k == MT_LAT - 1))
                ot = ost_p.tile([P, S], F32, tag="ost")
                if mo % 2 == 0:
                    nc.scalar.activation(ot[:], pt[:], AF.Identity,
                                         bias=b_out[:, mo:mo + 1])
                else:
                    nc.vector.tensor_scalar(ot[:], pt[:], b_out[:, mo:mo + 1],
                                            None, ALU.add)
                nc.sync.dma_start(out_ap[:, mo, :], ot[:])

    # Software-pipelined emission: sample i's LN/final work is emitted after
    # sample i+1's theta and first chunk, so its serial Vector chain hides
    # under ~48us of PE matmuls before its own final matmuls are due.
    for i in range(NSAMP):
        emit_theta(i)
        if i == 0:
            load_late_consts()
        emit_chunk(i, 0)
        if i > 0:
            emit_ln_out(i - 1)
        for cix in range(1, NCHUNK):
            emit_chunk(i, cix)
        emit_attn(i)
    emit_ln_out(NSAMP - 1)
    ctx.close()


_NC_CACHE = None


def _get_nc():
    global _NC_CACHE
    if _NC_CACHE is None:
        _NC_CACHE = build_nc()
    return _NC_CACHE


def kernel(st_feat, lt_feat, w_st, b_st, w_lt, b_lt, w_g, b_g,
           ln_w, ln_b, w_out, b_out):
    n = st_feat.shape[0]
    assert n == N_CORES * NSAMP
    bf16 = ml_dtypes.bfloat16
    st = np.asarray(st_feat, dtype=np.float32).reshape(n, C, S).astype(bf16)
    lt = np.asarray(lt_feat, dtype=np.float32).reshape(n, C, L).astype(bf16)
    w_stT = np.ascontiguousarray(np.asarray(w_st, np.float32).T).astype(bf16)
    w_ltT = np.ascontiguousarray(np.asarray(w_lt, np.float32).T).astype(bf16)
    w_gT = np.ascontiguousarray(np.asarray(w_g, np.float32).T).astype(bf16)
    w_outT = np.ascontiguousarray(np.asarray(w_out, np.float32).T).astype(bf16)
    shared = {
        "w_stT": w_stT, "w_ltT": w_ltT, "w_gT": w_gT, "w_outT": w_outT,
        "b_st": np.asarray(b_st, np.float32), "b_lt": np.asarray(b_lt, np.float32),
        "b_g": np.asarray(b_g, np.float32), "b_out": np.asarray(b_out, np.float32),
        "ln_w": np.ascontiguousarray(np.asarray(ln_w, np.float32)),
        "ln_b": np.ascontiguousarray(np.asarray(ln_b, np.float32)),
    }
    in_maps = []
    for c in range(N_CORES):
        sl = slice(c * NSAMP, (c + 1) * NSAMP)
        in_maps.append({"st": np.ascontiguousarray(st[sl]),
                        "lt": np.ascontiguousarray(lt[sl]), **shared})
    nc = _get_nc()
    res = bass_utils.run_bass_kernel_spmd(nc, in_maps, core_ids=list(range(N_CORES)))
    out = np.concatenate([res.results[c]["out"] for c in range(N_CORES)], axis=0)
    return out.reshape(n, C, S, 1, 1).astype(np.float32)


# revision 12
# speedup vs baseline: 1.0397x; 1.0211x over previous
"""Trainium2 Bass kernel for nn_NonLocalLayer (non-local attention block).

Data-parallel over batch: 32 samples -> 8 NeuronCores, 4 samples/core.
Per sample (all matmuls bf16 inputs, fp32 PSUM accumulation):
    theta = w_st @ st            (LAT=512, S=512)
    phi   = w_lt @ lt            (LAT=512, L=2048)
    gT    = (w_g @ lt)^T         (L=2048, LAT=512)   [computed transposed]
    scT   = phi^T @ theta        (L, S)              [scores transposed]
    E     = exp(scT / sqrt(LAT)) (no max-subtract; scores are O(1))
    D     = sum_L E              (1, S)
    U     = g @ E                (LAT, S)
    att   = U / D + b_g          (softmax-normalized attention output)
    LN over all (LAT, S), * ln_w + ln_b, relu
    out   = w_out @ y + b_out    (C=2048, S=512)
"""

import numpy as np
import ml_dtypes

import concourse.bacc as bacc
import concourse.mybir as mybir
import concourse.tile as tile
from concourse import bass_utils

N_CORES = 8
NSAMP = 4          # samples per core
C = 2048           # st/lt feature channels
LAT = 512          # latent channels
S = 512            # num st positions
L = 2048           # num lt positions
LN_EPS = 1e-5
P = 128
KT = C // P        # 16 contraction tiles
MT_LAT = LAT // P  # 4
MT_L = L // P      # 16
MT_C = C // P      # 16
NCHUNK = 4         # L chunks of 512
CHW = L // NCHUNK  # 512
INV_SQRT_LAT = 1.0 / float(np.sqrt(np.float32(LAT)))

BF = mybir.dt.bfloat16
F32 = mybir.dt.float32
AF = mybir.ActivationFunctionType
ALU = mybir.AluOpType


def build_nc():
    nc = bacc.Bacc("TRN2", target_bir_lowering=False, debug=False)

    st_d = nc.dram_tensor("st", (NSAMP, C, S), BF, kind="ExternalInput")
    lt_d = nc.dram_tensor("lt", (NSAMP, C, L), BF, kind="ExternalInput")
    w_stT_d = nc.dram_tensor("w_stT", (C, LAT), BF, kind="ExternalInput")
    w_ltT_d = nc.dram_tensor("w_ltT", (C, LAT), BF, kind="ExternalInput")
    w_gT_d = nc.dram_tensor("w_gT", (C, LAT), BF, kind="ExternalInput")
    w_outT_d = nc.dram_tensor("w_outT", (LAT, C), BF, kind="ExternalInput")
    b_st_d = nc.dram_tensor("b_st", (LAT,), F32, kind="ExternalInput")
    b_lt_d = nc.dram_tensor("b_lt", (LAT,), F32, kind="ExternalInput")
    b_g_d = nc.dram_tensor("b_g", (LAT,), F32, kind="ExternalInput")
    b_out_d = nc.dram_tensor("b_out", (C,), F32, kind="ExternalInput")
    ln_w_d = nc.dram_tensor("ln_w", (LAT, S), F32, kind="ExternalInput")
    ln_b_d = nc.dram_tensor("ln_b", (LAT, S), F32, kind="ExternalInput")
    out_d = nc.dram_tensor("out", (NSAMP, C, S), F32, kind="ExternalOutput")

    with tile.TileContext(nc) as tc:
        build_tile_kernel(
            tc, st_d, lt_d, w_stT_d, w_ltT_d, w_gT_d, w_outT_d,
            b_st_d, b_lt_d, b_g_d, b_out_d, ln_w_d, ln_b_d, out_d,
        )
    nc.finalize()
    return nc


def build_tile_kernel(tc, st_d, lt_d, w_stT_d, w_ltT_d, w_gT_d, w_outT_d,
                      b_st_d, b_lt_d, b_g_d, b_out_d, ln_w_d, ln_b_d, out_d):
    nc = tc.nc
    from contextlib import ExitStack
    ctx = ExitStack()
    consts = ctx.enter_context(tc.tile_pool(name="consts", bufs=1))
    st_p = ctx.enter_context(tc.tile_pool(name="st", bufs=4))
    th_p = ctx.enter_context(tc.tile_pool(name="theta", bufs=1))
    lt_p = ctx.enter_context(tc.tile_pool(name="lt", bufs=2))
    phi_p = ctx.enter_context(tc.tile_pool(name="phi", bufs=2))
    g_p = ctx.enter_context(tc.tile_pool(name="g", bufs=1))
    e_p = ctx.enter_context(tc.tile_pool(name="E", bufs=1))
    tmp_p = ctx.enter_context(tc.tile_pool(name="tmp", bufs=2))
    att_p = ctx.enter_context(tc.tile_pool(name="att", bufs=1))
    y_p = ctx.enter_context(tc.tile_pool(name="y", bufs=2))
    ost_p = ctx.enter_context(tc.tile_pool(name="ost", bufs=2))
    small_p = ctx.enter_context(tc.tile_pool(name="small", bufs=2))
    ps_w = ctx.enter_context(tc.tile_pool(name="ps_work", bufs=4, space="PSUM"))
    ps_u = ctx.enter_context(tc.tile_pool(name="ps_u", bufs=4, space="PSUM"))

    # ---- constants / weights (loaded once); ordered so the critical-path
    # weights (w_stT for theta, then w_ltT/w_gT for the chunk loop) arrive
    # first and the LN/output-stage constants load in the background.
    w_stT = consts.tile([P, KT, LAT], BF)
    w_ltT = consts.tile([P, KT, LAT], BF)
    w_gT = consts.tile([P, KT, LAT], BF)
    w_outT = consts.tile([P, MT_LAT, C], BF)
    b_st = consts.tile([P, MT_LAT], F32)
    b_lt = consts.tile([P, MT_LAT], F32)
    b_g = consts.tile([P, MT_LAT], F32)
    b_out = consts.tile([P, MT_C], F32)
    ln_w = consts.tile([P, MT_LAT, S], F32)
    ln_b = consts.tile([P, MT_LAT, S], F32)
    ones_col = consts.tile([P, 1], F32)
    eps_t = consts.tile([1, 1], F32)
    nc.sync.dma_start(w_stT[:], w_stT_d.ap().rearrange("(o p) m -> p o m", p=P))
    nc.sync.dma_start(b_st[:], b_st_d.ap().rearrange("(o p) -> p o", p=P))
    nc.sync.dma_start(w_ltT[:], w_ltT_d.ap().rearrange("(o p) m -> p o m", p=P))
    nc.sync.dma_start(w_gT[:], w_gT_d.ap().rearrange("(o p) m -> p o m", p=P))
    nc.sync.dma_start(b_lt[:], b_lt_d.ap().rearrange("(o p) -> p o", p=P))
    nc.sync.dma_start(b_g[:], b_g_d.ap().rearrange("(o p) -> p o", p=P))
    nc.vector.memset(ones_col[:], 1.0)
    nc.vector.memset(eps_t[:], LN_EPS)

    def load_late_consts():
        nc.sync.dma_start(w_outT[:], w_outT_d.ap().rearrange("(o p) m -> p o m", p=P))
        nc.sync.dma_start(ln_w[:], ln_w_d.ap().rearrange("(o p) s -> p o s", p=P))
        nc.sync.dma_start(ln_b[:], ln_b_d.ap().rearrange("(o p) s -> p o s", p=P))
        nc.sync.dma_start(b_out[:], b_out_d.ap().rearrange("(o p) -> p o", p=P))

    # Per-sample state carried between emission stages
    state = {}

    def emit_theta(i):
        # st is streamed per k-tile; each tile is reused by the 4 m-tiles, so
        # the m-loop is innermost here (theta psum banks accumulate in turn).
        st_ap = st_d.ap()[i].rearrange("(o p) s -> p o s", p=P)
        theta = th_p.tile([P, MT_LAT, S], BF, tag="theta")
        with nc.named_scope(f"theta{i}"):
            pts = [ps_w.tile([P, S], F32, tag="work", name=f"pth{m}")
                   for m in range(MT_LAT)]
            for k in range(KT):
                st_sb = st_p.tile([P, S], BF, tag="st", name="st_sb")
                nc.sync.dma_start(st_sb[:], st_ap[:, k, :])
                for m in range(MT_LAT):
                    nc.tensor.matmul(pts[m][:], w_stT[:, k, m * P:(m + 1) * P],
                                     st_sb[:], start=(k == 0), stop=(k == KT - 1))
            for m in range(MT_LAT):
                nc.scalar.activation(theta[:, m, :], pts[m][:], AF.Identity,
                                     bias=b_st[:, m:m + 1])
        state[i] = {"theta": theta}

    def emit_chunk(i, cix):
        sti = state[i]
        theta = sti["theta"]
        if cix == 0:
            sti["g"] = g_p.tile([P, MT_L, LAT], BF, tag="g", name="g_sb")
            sti["E"] = e_p.tile([P, MT_L, S], BF, tag="E", name="e_sb")
            sti["dacc"] = tmp_p.tile([P, S], F32, tag="dacc", name="dacc")
        g_sb, e_sb, dacc = sti["g"], sti["E"], sti["dacc"]
        with nc.named_scope(f"chunks{i}"):
            if True:
                lt_sb = lt_p.tile([P, KT, CHW], BF, tag="lt")
                nc.sync.dma_start(
                    lt_sb[:],
                    lt_d.ap()[i, :, cix * CHW:(cix + 1) * CHW]
                    .rearrange("(o p) l -> p o l", p=P))
                # phi (LAT x CHW) for this chunk
                phi_sb = phi_p.tile([P, MT_LAT, CHW], BF, tag="phi")
                for m in range(MT_LAT):
                    pt = ps_w.tile([P, CHW], F32, tag="work")
                    for k in range(KT):
                        nc.tensor.matmul(pt[:], w_ltT[:, k, m * P:(m + 1) * P],
                                         lt_sb[:, k, :], start=(k == 0),
                                         stop=(k == KT - 1))
                    nc.vector.tensor_scalar(phi_sb[:, m, :], pt[:],
                                            b_lt[:, m:m + 1], None, ALU.add)
                # gT (CHW x LAT), 4 L-part tiles
                for j in range(MT_LAT):
                    lk = cix * MT_LAT + j
                    pt = ps_w.tile([P, LAT], F32, tag="work")
                    for k in range(KT):
                        nc.tensor.matmul(pt[:], lt_sb[:, k, j * P:(j + 1) * P],
                                         w_gT[:, k, :], start=(k == 0),
                                         stop=(k == KT - 1))
                    nc.vector.tensor_copy(g_sb[:, lk, :], pt[:])
                # scores^T (CHW x S) then E = exp(sc/sqrt(LAT))
                for j in range(MT_LAT):
                    lk = cix * MT_LAT + j
                    pt = ps_w.tile([P, S], F32, tag="work")
                    for m in range(MT_LAT):
                        nc.tensor.matmul(pt[:], phi_sb[:, m, j * P:(j + 1) * P],
                                         theta[:, m, :], start=(m == 0),
                                         stop=(m == MT_LAT - 1))
                    nc.scalar.activation(e_sb[:, lk, :], pt[:], AF.Exp,
                                         scale=INV_SQRT_LAT)
                    if lk == 0:
                        nc.vector.tensor_copy(dacc[:], e_sb[:, 0, :])
                    else:
                        nc.vector.tensor_tensor(dacc[:], dacc[:], e_sb[:, lk, :],
                                                ALU.add)

    def emit_attn(i):
        sti = state[i]
        g_sb, e_sb, dacc = sti["g"], sti["E"], sti["dacc"]
        with nc.named_scope(f"attn{i}"):
            psu = []
            for m in range(MT_LAT):
                pu = ps_u.tile([P, S], F32, tag="u")
                for lk in range(MT_L):
                    nc.tensor.matmul(pu[:], g_sb[:, lk, m * P:(m + 1) * P],
                                     e_sb[:, lk, :], start=(lk == 0),
                                     stop=(lk == MT_L - 1))
                psu.append(pu)
            # D = column sums of dacc across partitions (1 x S), fp32 matmul
            pd = ps_w.tile([1, S], F32, tag="work")
            nc.tensor.matmul(pd[:], ones_col[:], dacc[:], start=True, stop=True)
            r_sb = small_p.tile([1, S], F32, tag="r")
            nc.vector.reciprocal(r_sb[:], pd[:])
            rb = tmp_p.tile([P, S], F32, tag="rb")
            nc.gpsimd.partition_broadcast(rb[:], r_sb[:])
            att = att_p.tile([P, MT_LAT, S], F32, tag="att")
            for m in range(MT_LAT):
                nc.vector.tensor_tensor(att[:, m, :], psu[m][:], rb[:], ALU.mult)
                nc.vector.tensor_scalar(att[:, m, :], att[:, m, :],
                                        b_g[:, m:m + 1], None, ALU.add)
        sti["att"] = att

    def emit_ln(i):
        sti = state[i]
        att = sti["att"]
        with nc.named_scope(f"ln{i}"):
            # per-partition stats over the 4*S free elems
            stats = small_p.tile([P, MT_LAT, nc.vector.BN_STATS_DIM], F32, tag="bns")
            for m in range(MT_LAT):
                nc.vector.bn_stats(stats[:, m, :], att[:, m, :])
            mv = small_p.tile([P, nc.vector.BN_AGGR_DIM], F32, tag="bnm")
            nc.vector.bn_aggr(mv[:], stats[:])
            # pack [mean_p, mean_p^2 + var_p] then reduce across partitions
            t2 = small_p.tile([P, 2], F32, tag="t2")
            nc.vector.tensor_copy(t2[:, 0:1], mv[:, 0:1])
            nc.vector.tensor_tensor(t2[:, 1:2], mv[:, 0:1], mv[:, 0:1], ALU.mult)
            nc.vector.tensor_tensor(t2[:, 1:2], t2[:, 1:2], mv[:, 1:2], ALU.add)
            psm = ps_w.tile([1, 2], F32, tag="work")
            nc.tensor.matmul(psm[:], ones_col[:], t2[:], start=True, stop=True)
            sg = small_p.tile([1, 4], F32, tag="sg")
            # sg[0]=mu, sg[1]=E[x^2]
            nc.scalar.mul(sg[:, 0:2], psm[:], 1.0 / P)
            # var = E[x^2] - mu^2 -> sg[2]
            nc.vector.tensor_tensor(sg[:, 2:3], sg[:, 0:1], sg[:, 0:1], ALU.mult)
            nc.vector.tensor_tensor(sg[:, 2:3], sg[:, 1:2], sg[:, 2:3], ALU.subtract)
            # rstd = 1/sqrt(var + eps) -> sg[3]
            nc.scalar.activation(sg[:, 3:4], sg[:, 2:3], AF.Sqrt, bias=eps_t[:])
            nc.vector.reciprocal(sg[:, 3:4], sg[:, 3:4])
            musd = small_p.tile([1, 2], F32, tag="musd")
            nc.vector.tensor_copy(musd[:, 0:1], sg[:, 0:1])
            nc.vector.tensor_copy(musd[:, 1:2], sg[:, 3:4])
            musd_b = small_p.tile([P, 2], F32, tag="musdb")
            nc.gpsimd.partition_broadcast(musd_b[:], musd[:])
            y_sb = y_p.tile([P, MT_LAT, S], BF, tag="y")
            for m in range(MT_LAT):
                t = tmp_p.tile([P, S], F32, tag="lnt")
                nc.vector.tensor_scalar(t[:], att[:, m, :], musd_b[:, 0:1],
                                        musd_b[:, 1:2], ALU.subtract, ALU.mult)
                nc.vector.tensor_tensor(t[:], t[:], ln_w[:, m, :], ALU.mult)
                nc.vector.tensor_tensor(t[:], t[:], ln_b[:, m, :], ALU.add)
                nc.scalar.activation(y_sb[:, m, :], t[:], AF.Relu)
        sti["y"] = y_sb

    def emit_out(i):
        sti = state[i]
        y_sb = sti["y"]
        with nc.named_scope(f"out{i}"):
            out_ap = out_d.ap()[i].rearrange("(o p) s -> p o s", p=P)
            for mo in range(MT_C):
                pool = ps_w if mo % 2 == 0 else ps_u
                pt = pool.tile([P, S], F32, tag="work" if mo % 2 == 0 else "u")
                for k in range(MT_LAT):
                    nc.tensor.matmul(pt[:], w_outT[:, k, mo * P:(mo + 1) * P],
                                     y_sb[:, k, :], start=(k == 0),
                                     stop=(k == MT_LAT - 1))
                ot = ost_p.tile([P, S], F32, tag="ost")
                if mo % 2 == 0:
                    nc.scalar.activation(ot[:], pt[:], AF.Identity,
                                         bias=b_out[:, mo:mo + 1])
                else:
                    nc.vector.tensor_scalar(ot[:], pt[:], b_out[:, mo:mo + 1],
                                            None, ALU.add)
                nc.sync.dma_start(out_ap[:, mo, :], ot[:])

    # Software-pipelined emission. Sample i's LN chain (Vector-serial) is
    # emitted right after sample i+1's theta so it runs at the front of the
    # Vector queue while the PE does theta + chunk0 (~48us); the final matmuls
    # follow chunk0. Sample 0's final matmuls are held back to the tail where
    # they cover sample 3's LN chain latency.
    for i in range(NSAMP):
        emit_theta(i)
        if i > 0:
            emit_ln(i - 1)
        emit_chunk(i, 0)
        if i == 0:
            load_late_consts()
        if i > 1:
            emit_out(i - 1)
        for cix in range(1, NCHUNK):
            emit_chunk(i, cix)
        emit_attn(i)
    emit_ln(NSAMP - 1)
    emit_out(0)
    emit_out(NSAMP - 1)
    ctx.close()


_NC_CACHE = None


def _get_nc():
    global _NC_CACHE
    if _NC_CACHE is None:
        _NC_CACHE = build_nc()
    return _NC_CACHE


def kernel(st_feat, lt_feat, w_st, b_st, w_lt, b_lt, w_g, b_g,
           ln_w, ln_b, w_out, b_out):
    n = st_feat.shape[0]
    assert n == N_CORES * NSAMP
    bf16 = ml_dtypes.bfloat16
    st = np.asarray(st_feat, dtype=np.float32).reshape(n, C, S).astype(bf16)
    lt = np.asarray(lt_feat, dtype=np.float32).reshape(n, C, L).astype(bf16)
    w_stT = np.ascontiguousarray(np.asarray(w_st, np.float32).T).astype(bf16)
    w_ltT = np.ascontiguousarray(np.asarray(w_lt, np.float32).T).astype(bf16)
    w_gT = np.ascontiguousarray(np.asarray(w_g, np.float32).T).astype(bf16)
    w_outT = np.ascontiguousarray(np.asarray(w_out, np.float32).T).astype(bf16)
    shared = {
        "w_stT": w_stT, "w_ltT": w_ltT, "w_gT": w_gT, "w_outT": w_outT,
        "b_st": np.asarray(b_st, np.float32), "b_lt": np.asarray(b_lt, np.float32),
        "b_g": np.asarray(b_g, np.float32), "b_out": np.asarray(b_out, np.float32),
        "ln_w": np.ascontiguousarray(np.asarray(ln_w, np.float32)),
        "ln_b": np.ascontiguousarray(np.asarray(ln_b, np.float32)),
    }
    in_maps = []
    for c in range(N_CORES):
        sl = slice(c * NSAMP, (c + 1) * NSAMP)
        in_maps.append({"st": np.ascontiguousarray(st[sl]),
                        "lt": np.ascontiguousarray(lt[sl]), **shared})
    nc = _get_nc()
    res = bass_utils.run_bass_kernel_spmd(nc, in_maps, core_ids=list(range(N_CORES)))
    out = np.concatenate([res.results[c]["out"] for c in range(N_CORES)], axis=0)
    return out.reshape(n, C, S, 1, 1).astype(np.float32)


# revision 13
# speedup vs baseline: 1.0567x; 1.0164x over previous
"""Trainium2 Bass kernel for nn_NonLocalLayer (non-local attention block).

Data-parallel over batch: 32 samples -> 8 NeuronCores, 4 samples/core.
Per sample (all matmuls bf16 inputs, fp32 PSUM accumulation):
    theta = w_st @ st            (LAT=512, S=512)
    phi   = w_lt @ lt            (LAT=512, L=2048)
    gT    = (w_g @ lt)^T         (L=2048, LAT=512)   [computed transposed]
    scT   = phi^T @ theta        (L, S)              [scores transposed]
    E     = exp(scT / sqrt(LAT)) (no max-subtract; scores are O(1))
    D     = sum_L E              (1, S)
    U     = g @ E                (LAT, S)
    att   = U / D + b_g          (softmax-normalized attention output)
    LN over all (LAT, S), * ln_w + ln_b, relu
    out   = w_out @ y + b_out    (C=2048, S=512)
"""

import numpy as np
import ml_dtypes

import concourse.bacc as bacc
import concourse.mybir as mybir
import concourse.tile as tile
from concourse import bass_utils

N_CORES = 8
NSAMP = 4          # samples per core
C = 2048           # st/lt feature channels
LAT = 512          # latent channels
S = 512            # num st positions
L = 2048           # num lt positions
LN_EPS = 1e-5
P = 128
KT = C // P        # 16 contraction tiles
MT_LAT = LAT // P  # 4
MT_L = L // P      # 16
MT_C = C // P      # 16
NCHUNK = 4         # L chunks of 512
CHW = L // NCHUNK  # 512
INV_SQRT_LAT = 1.0 / float(np.sqrt(np.float32(LAT)))

BF = mybir.dt.bfloat16
F32 = mybir.dt.float32
AF = mybir.ActivationFunctionType
ALU = mybir.AluOpType


def build_nc():
    nc = bacc.Bacc("TRN2", target_bir_lowering=False, debug=False)

    st_d = nc.dram_tensor("st", (NSAMP, C, S), BF, kind="ExternalInput")
    lt_d = nc.dram_tensor("lt", (NSAMP, C, L), BF, kind="ExternalInput")
    w_stT_d = nc.dram_tensor("w_stT", (C, LAT), BF, kind="ExternalInput")
    w_ltT_d = nc.dram_tensor("w_ltT", (C, LAT), BF, kind="ExternalInput")
    w_gT_d = nc.dram_tensor("w_gT", (C, LAT), BF, kind="ExternalInput")
    w_outT_d = nc.dram_tensor("w_outT", (LAT, C), BF, kind="ExternalInput")
    b_st_d = nc.dram_tensor("b_st", (LAT,), F32, kind="ExternalInput")
    b_lt_d = nc.dram_tensor("b_lt", (LAT,), F32, kind="ExternalInput")
    b_g_d = nc.dram_tensor("b_g", (LAT,), F32, kind="ExternalInput")
    b_out_d = nc.dram_tensor("b_out", (C,), F32, kind="ExternalInput")
    ln_w_d = nc.dram_tensor("ln_w", (LAT, S), F32, kind="ExternalInput")
    ln_b_d = nc.dram_tensor("ln_b", (LAT, S), F32, kind="ExternalInput")
    out_d = nc.dram_tensor("out", (NSAMP, C, S), F32, kind="ExternalOutput")

    with tile.TileContext(nc) as tc:
        build_tile_kernel(
            tc, st_d, lt_d, w_stT_d, w_ltT_d, w_gT_d, w_outT_d,
            b_st_d, b_lt_d, b_g_d, b_out_d, ln_w_d, ln_b_d, out_d,
        )
    nc.finalize()
    return nc


def build_tile_kernel(tc, st_d, lt_d, w_stT_d, w_ltT_d, w_gT_d, w_outT_d,
                      b_st_d, b_lt_d, b_g_d, b_out_d, ln_w_d, ln_b_d, out_d):
    nc = tc.nc
    from contextlib import ExitStack
    ctx = ExitStack()
    consts = ctx.enter_context(tc.tile_pool(name="consts", bufs=1))
    st_p = ctx.enter_context(tc.tile_pool(name="st", bufs=4))
    th_p = ctx.enter_context(tc.tile_pool(name="theta", bufs=1))
    lt_p = ctx.enter_context(tc.tile_pool(name="lt", bufs=2))
    phi_p = ctx.enter_context(tc.tile_pool(name="phi", bufs=2))
    g_p = ctx.enter_context(tc.tile_pool(name="g", bufs=1))
    e_p = ctx.enter_context(tc.tile_pool(name="E", bufs=1))
    tmp_p = ctx.enter_context(tc.tile_pool(name="tmp", bufs=2))
    att_p = ctx.enter_context(tc.tile_pool(name="att", bufs=1))
    y_p = ctx.enter_context(tc.tile_pool(name="y", bufs=2))
    ost_p = ctx.enter_context(tc.tile_pool(name="ost", bufs=4))
    small_p = ctx.enter_context(tc.tile_pool(name="small", bufs=2))
    ps_w = ctx.enter_context(tc.tile_pool(name="ps_work", bufs=4, space="PSUM"))
    ps_u = ctx.enter_context(tc.tile_pool(name="ps_u", bufs=4, space="PSUM"))

    # ---- constants / weights (loaded once); ordered so the critical-path
    # weights (w_stT for theta, then w_ltT/w_gT for the chunk loop) arrive
    # first and the LN/output-stage constants load in the background.
    w_stT = consts.tile([P, KT, LAT], BF)
    w_ltT = consts.tile([P, KT, LAT], BF)
    w_gT = consts.tile([P, KT, LAT], BF)
    w_outT = consts.tile([P, MT_LAT, C], BF)
    b_st = consts.tile([P, MT_LAT], F32)
    b_lt = consts.tile([P, MT_LAT], F32)
    b_g = consts.tile([P, MT_LAT], F32)
    b_out = consts.tile([P, MT_C], F32)
    ln_w = consts.tile([P, MT_LAT, S], F32)
    ln_b = consts.tile([P, MT_LAT, S], F32)
    ones_col = consts.tile([P, 1], F32)
    eps_t = consts.tile([1, 1], F32)
    # Sliced weight loads: the first k-slices arrive quickly so theta/chunk
    # matmuls can start while the rest streams in.
    _w_stT_src = w_stT_d.ap().rearrange("(o p) m -> p o m", p=P)
    _w_ltT_src = w_ltT_d.ap().rearrange("(o p) m -> p o m", p=P)
    _w_gT_src = w_gT_d.ap().rearrange("(o p) m -> p o m", p=P)
    nc.sync.dma_start(w_stT[:, 0:4, :], _w_stT_src[:, 0:4, :])
    nc.sync.dma_start(b_st[:], b_st_d.ap().rearrange("(o p) -> p o", p=P))
    for q in range(1, 4):
        nc.sync.dma_start(w_stT[:, 4 * q:4 * (q + 1), :], _w_stT_src[:, 4 * q:4 * (q + 1), :])
    for q in range(4):
        nc.sync.dma_start(w_ltT[:, 4 * q:4 * (q + 1), :], _w_ltT_src[:, 4 * q:4 * (q + 1), :])
        nc.sync.dma_start(w_gT[:, 4 * q:4 * (q + 1), :], _w_gT_src[:, 4 * q:4 * (q + 1), :])
    nc.sync.dma_start(b_lt[:], b_lt_d.ap().rearrange("(o p) -> p o", p=P))
    nc.sync.dma_start(b_g[:], b_g_d.ap().rearrange("(o p) -> p o", p=P))
    nc.vector.memset(ones_col[:], 1.0)
    nc.vector.memset(eps_t[:], LN_EPS)

    def load_late_consts():
        nc.sync.dma_start(w_outT[:], w_outT_d.ap().rearrange("(o p) m -> p o m", p=P))
        nc.sync.dma_start(ln_w[:], ln_w_d.ap().rearrange("(o p) s -> p o s", p=P))
        nc.sync.dma_start(ln_b[:], ln_b_d.ap().rearrange("(o p) s -> p o s", p=P))
        nc.sync.dma_start(b_out[:], b_out_d.ap().rearrange("(o p) -> p o", p=P))

    # Per-sample state carried between emission stages
    state = {}

    def emit_theta(i):
        # st is streamed per k-tile; each tile is reused by the 4 m-tiles, so
        # the m-loop is innermost here (theta psum banks accumulate in turn).
        st_ap = st_d.ap()[i].rearrange("(o p) s -> p o s", p=P)
        theta = th_p.tile([P, MT_LAT, S], BF, tag="theta")
        with nc.named_scope(f"theta{i}"):
            pts = [ps_w.tile([P, S], F32, tag="work", name=f"pth{m}")
                   for m in range(MT_LAT)]
            for k in range(KT):
                st_sb = st_p.tile([P, S], BF, tag="st", name="st_sb")
                nc.sync.dma_start(st_sb[:], st_ap[:, k, :])
                for m in range(MT_LAT):
                    nc.tensor.matmul(pts[m][:], w_stT[:, k, m * P:(m + 1) * P],
                                     st_sb[:], start=(k == 0), stop=(k == KT - 1))
            for m in range(MT_LAT):
                nc.scalar.activation(theta[:, m, :], pts[m][:], AF.Identity,
                                     bias=b_st[:, m:m + 1])
        state[i] = {"theta": theta}

    def emit_chunk(i, cix):
        sti = state[i]
        theta = sti["theta"]
        if cix == 0:
            sti["g"] = g_p.tile([P, MT_L, LAT], BF, tag="g", name="g_sb")
            sti["E"] = e_p.tile([P, MT_L, S], BF, tag="E", name="e_sb")
            sti["dacc"] = tmp_p.tile([P, S], F32, tag="dacc", name="dacc")
        g_sb, e_sb, dacc = sti["g"], sti["E"], sti["dacc"]
        with nc.named_scope(f"chunks{i}"):
            if True:
                lt_sb = lt_p.tile([P, KT, CHW], BF, tag="lt")
                nc.sync.dma_start(
                    lt_sb[:],
                    lt_d.ap()[i, :, cix * CHW:(cix + 1) * CHW]
                    .rearrange("(o p) l -> p o l", p=P))
                # phi (LAT x CHW) for this chunk
                phi_sb = phi_p.tile([P, MT_LAT, CHW], BF, tag="phi")
                for m in range(MT_LAT):
                    pt = ps_w.tile([P, CHW], F32, tag="work")
                    for k in range(KT):
                        nc.tensor.matmul(pt[:], w_ltT[:, k, m * P:(m + 1) * P],
                                         lt_sb[:, k, :], start=(k == 0),
                                         stop=(k == KT - 1))
                    nc.vector.tensor_scalar(phi_sb[:, m, :], pt[:],
                                            b_lt[:, m:m + 1], None, ALU.add)
                # gT (CHW x LAT), 4 L-part tiles
                for j in range(MT_LAT):
                    lk = cix * MT_LAT + j
                    pt = ps_w.tile([P, LAT], F32, tag="work")
                    for k in range(KT):
                        nc.tensor.matmul(pt[:], lt_sb[:, k, j * P:(j + 1) * P],
                                         w_gT[:, k, :], start=(k == 0),
                                         stop=(k == KT - 1))
                    nc.vector.tensor_copy(g_sb[:, lk, :], pt[:])
                # scores^T (CHW x S) then E = exp(sc/sqrt(LAT))
                for j in range(MT_LAT):
                    lk = cix * MT_LAT + j
                    pt = ps_w.tile([P, S], F32, tag="work")
                    for m in range(MT_LAT):
                        nc.tensor.matmul(pt[:], phi_sb[:, m, j * P:(j + 1) * P],
                                         theta[:, m, :], start=(m == 0),
                                         stop=(m == MT_LAT - 1))
                    nc.scalar.activation(e_sb[:, lk, :], pt[:], AF.Exp,
                                         scale=INV_SQRT_LAT)
                    if lk == 0:
                        nc.vector.tensor_copy(dacc[:], e_sb[:, 0, :])
                    else:
                        nc.vector.tensor_tensor(dacc[:], dacc[:], e_sb[:, lk, :],
                                                ALU.add)

    def emit_attn(i):
        sti = state[i]
        g_sb, e_sb, dacc = sti["g"], sti["E"], sti["dacc"]
        with nc.named_scope(f"attn{i}"):
            psu = []
            for m in range(MT_LAT):
                pu = ps_u.tile([P, S], F32, tag="u")
                for lk in range(MT_L):
                    nc.tensor.matmul(pu[:], g_sb[:, lk, m * P:(m + 1) * P],
                                     e_sb[:, lk, :], start=(lk == 0),
                                     stop=(lk == MT_L - 1))
                psu.append(pu)
            # D = column sums of dacc across partitions (1 x S), fp32 matmul
            pd = ps_w.tile([1, S], F32, tag="work")
            nc.tensor.matmul(pd[:], ones_col[:], dacc[:], start=True, stop=True)
            r_sb = small_p.tile([1, S], F32, tag="r")
            nc.vector.reciprocal(r_sb[:], pd[:])
            rb = tmp_p.tile([P, S], F32, tag="rb")
            nc.gpsimd.partition_broadcast(rb[:], r_sb[:])
            att = att_p.tile([P, MT_LAT, S], F32, tag="att")
            for m in range(MT_LAT):
                nc.vector.tensor_tensor(att[:, m, :], psu[m][:], rb[:], ALU.mult)
                nc.vector.tensor_scalar(att[:, m, :], att[:, m, :],
                                        b_g[:, m:m + 1], None, ALU.add)
        sti["att"] = att

    def emit_ln(i):
        sti = state[i]
        att = sti["att"]
        with nc.named_scope(f"ln{i}"):
            # per-partition stats over the 4*S free elems
            stats = small_p.tile([P, MT_LAT, nc.vector.BN_STATS_DIM], F32, tag="bns")
            for m in range(MT_LAT):
                nc.vector.bn_stats(stats[:, m, :], att[:, m, :])
            mv = small_p.tile([P, nc.vector.BN_AGGR_DIM], F32, tag="bnm")
            nc.vector.bn_aggr(mv[:], stats[:])
            # pack [mean_p, mean_p^2 + var_p] then reduce across partitions
            t2 = small_p.tile([P, 2], F32, tag="t2")
            nc.vector.tensor_copy(t2[:, 0:1], mv[:, 0:1])
            nc.vector.tensor_tensor(t2[:, 1:2], mv[:, 0:1], mv[:, 0:1], ALU.mult)
            nc.vector.tensor_tensor(t2[:, 1:2], t2[:, 1:2], mv[:, 1:2], ALU.add)
            psm = ps_w.tile([1, 2], F32, tag="work")
            nc.tensor.matmul(psm[:], ones_col[:], t2[:], start=True, stop=True)
            sg = small_p.tile([1, 4], F32, tag="sg")
            # sg[0]=mu, sg[1]=E[x^2]
            nc.scalar.mul(sg[:, 0:2], psm[:], 1.0 / P)
            # var = E[x^2] - mu^2 -> sg[2]
            nc.vector.tensor_tensor(sg[:, 2:3], sg[:, 0:1], sg[:, 0:1], ALU.mult)
            nc.vector.tensor_tensor(sg[:, 2:3], sg[:, 1:2], sg[:, 2:3], ALU.subtract)
            # rstd = 1/sqrt(var + eps) -> sg[3]
            nc.scalar.activation(sg[:, 3:4], sg[:, 2:3], AF.Sqrt, bias=eps_t[:])
            nc.vector.reciprocal(sg[:, 3:4], sg[:, 3:4])
            musd = small_p.tile([1, 2], F32, tag="musd")
            nc.vector.tensor_copy(musd[:, 0:1], sg[:, 0:1])
            nc.vector.tensor_copy(musd[:, 1:2], sg[:, 3:4])
            musd_b = small_p.tile([P, 2], F32, tag="musdb")
            nc.gpsimd.partition_broadcast(musd_b[:], musd[:])
            y_sb = y_p.tile([P, MT_LAT, S], BF, tag="y")
            for m in range(MT_LAT):
                t = tmp_p.tile([P, S], F32, tag="lnt")
                nc.vector.tensor_scalar(t[:], att[:, m, :], musd_b[:, 0:1],
                                        musd_b[:, 1:2], ALU.subtract, ALU.mult)
                nc.vector.tensor_tensor(t[:], t[:], ln_w[:, m, :], ALU.mult)
                nc.vector.tensor_tensor(t[:], t[:], ln_b[:, m, :], ALU.add)
                nc.scalar.activation(y_sb[:, m, :], t[:], AF.Relu)
        sti["y"] = y_sb

    def emit_out(i):
        sti = state[i]
        y_sb = sti["y"]
        with nc.named_scope(f"out{i}"):
            out_ap = out_d.ap()[i].rearrange("(o p) s -> p o s", p=P)
            for mo in range(MT_C):
                pool = ps_w if mo % 2 == 0 else ps_u
                pt = pool.tile([P, S], F32, tag="work" if mo % 2 == 0 else "u")
                for k in range(MT_LAT):
                    nc.tensor.matmul(pt[:], w_outT[:, k, mo * P:(mo + 1) * P],
                                     y_sb[:, k, :], start=(k == 0),
                                     stop=(k == MT_LAT - 1))
                ot = ost_p.tile([P, S], F32, tag="ost")
                if mo % 2 == 0:
                    nc.scalar.activation(ot[:], pt[:], AF.Identity,
                                         bias=b_out[:, mo:mo + 1])
                else:
                    nc.vector.tensor_scalar(ot[:], pt[:], b_out[:, mo:mo + 1],
                                            None, ALU.add)
                nc.sync.dma_start(out_ap[:, mo, :], ot[:])

    # Software-pipelined emission. Sample i's LN chain (Vector-serial) is
    # emitted right after sample i+1's theta so it runs at the front of the
    # Vector queue while the PE does theta + chunk0 (~48us); the final matmuls
    # follow chunk0. Sample 0's final matmuls are held back to the tail where
    # they cover sample 3's LN chain latency.
    for i in range(NSAMP):
        emit_theta(i)
        if i > 0:
            emit_ln(i - 1)
        emit_chunk(i, 0)
        if i == 0:
            load_late_consts()
        if i > 1:
            emit_out(i - 1)
        for cix in range(1, NCHUNK):
            emit_chunk(i, cix)
        emit_attn(i)
    emit_out(0)
    emit_ln(NSAMP - 1)
    emit_out(NSAMP - 1)
    ctx.close()


_NC_CACHE = None


def _get_nc():
    global _NC_CACHE
    if _NC_CACHE is None:
        _NC_CACHE = build_nc()
    return _NC_CACHE


def kernel(st_feat, lt_feat, w_st, b_st, w_lt, b_lt, w_g, b_g,
           ln_w, ln_b, w_out, b_out):
    n = st_feat.shape[0]
    assert n == N_CORES * NSAMP
    bf16 = ml_dtypes.bfloat16
    st = np.asarray(st_feat, dtype=np.float32).reshape(n, C, S).astype(bf16)
    lt = np.asarray(lt_feat, dtype=np.float32).reshape(n, C, L).astype(bf16)
    w_stT = np.ascontiguousarray(np.asarray(w_st, np.float32).T).astype(bf16)
    w_ltT = np.ascontiguousarray(np.asarray(w_lt, np.float32).T).astype(bf16)
    w_gT = np.ascontiguousarray(np.asarray(w_g, np.float32).T).astype(bf16)
    w_outT = np.ascontiguousarray(np.asarray(w_out, np.float32).T).astype(bf16)
    shared = {
        "w_stT": w_stT, "w_ltT": w_ltT, "w_gT": w_gT, "w_outT": w_outT,
        "b_st": np.asarray(b_st, np.float32), "b_lt": np.asarray(b_lt, np.float32),
        "b_g": np.asarray(b_g, np.float32), "b_out": np.asarray(b_out, np.float32),
        "ln_w": np.ascontiguousarray(np.asarray(ln_w, np.float32)),
        "ln_b": np.ascontiguousarray(np.asarray(ln_b, np.float32)),
    }
    in_maps = []
    for c in range(N_CORES):
        sl = slice(c * NSAMP, (c + 1) * NSAMP)
        in_maps.append({"st": np.ascontiguousarray(st[sl]),
                        "lt": np.ascontiguousarray(lt[sl]), **shared})
    nc = _get_nc()
    res = bass_utils.run_bass_kernel_spmd(nc, in_maps, core_ids=list(range(N_CORES)))
    out = np.concatenate([res.results[c]["out"] for c in range(N_CORES)], axis=0)
    return out.reshape(n, C, S, 1, 1).astype(np.float32)


# revision 16
# speedup vs baseline: 1.0888x; 1.0304x over previous
"""Trainium2 Bass kernel for nn_NonLocalLayer (non-local attention block).

Data-parallel over batch: 32 samples -> 8 NeuronCores, 4 samples/core.
Per sample (all matmuls bf16 inputs, fp32 PSUM accumulation):
    theta = w_st @ st            (LAT=512, S=512)
    phi   = w_lt @ lt            (LAT=512, L=2048)
    gT    = (w_g @ lt)^T         (L=2048, LAT=512)   [computed transposed]
    scT   = phi^T @ theta        (L, S)              [scores transposed]
    E     = exp(scT / sqrt(LAT)) (no max-subtract; scores are O(1))
    D     = sum_L E              (1, S)
    U     = g @ E                (LAT, S)
    att   = U / D + b_g          (softmax-normalized attention output)
    LN over all (LAT, S), * ln_w + ln_b, relu
    out   = w_out @ y + b_out    (C=2048, S=512)
"""

import numpy as np
import ml_dtypes

import concourse.bacc as bacc
import concourse.mybir as mybir
import concourse.tile as tile
from concourse import bass_utils

N_CORES = 8
NSAMP = 4          # samples per core
C = 2048           # st/lt feature channels
LAT = 512          # latent channels
S = 512            # num st positions
L = 2048           # num lt positions
LN_EPS = 1e-5
P = 128
KT = C // P        # 16 contraction tiles
MT_LAT = LAT // P  # 4
MT_L = L // P      # 16
MT_C = C // P      # 16
NCHUNK = 4         # L chunks of 512
CHW = L // NCHUNK  # 512
INV_SQRT_LAT = 1.0 / float(np.sqrt(np.float32(LAT)))

BF = mybir.dt.bfloat16
F32 = mybir.dt.float32
AF = mybir.ActivationFunctionType
ALU = mybir.AluOpType


def build_nc():
    nc = bacc.Bacc("TRN2", target_bir_lowering=False, debug=False)

    st_d = nc.dram_tensor("st", (NSAMP, C, S), BF, kind="ExternalInput")
    lt_d = nc.dram_tensor("lt", (NSAMP, C, L), BF, kind="ExternalInput")
    w_stT_d = nc.dram_tensor("w_stT", (C, LAT), BF, kind="ExternalInput")
    w_ltT_d = nc.dram_tensor("w_ltT", (C, LAT), BF, kind="ExternalInput")
    w_gT_d = nc.dram_tensor("w_gT", (C, LAT), BF, kind="ExternalInput")
    w_outT_d = nc.dram_tensor("w_outT", (LAT, C), BF, kind="ExternalInput")
    b_st_d = nc.dram_tensor("b_st", (LAT,), F32, kind="ExternalInput")
    b_lt_d = nc.dram_tensor("b_lt", (LAT,), F32, kind="ExternalInput")
    b_g_d = nc.dram_tensor("b_g", (LAT,), F32, kind="ExternalInput")
    b_out_d = nc.dram_tensor("b_out", (C,), F32, kind="ExternalInput")
    ln_w_d = nc.dram_tensor("ln_w", (LAT, S), F32, kind="ExternalInput")
    ln_b_d = nc.dram_tensor("ln_b", (LAT, S), F32, kind="ExternalInput")
    out_d = nc.dram_tensor("out", (NSAMP, C, S), F32, kind="ExternalOutput")

    with tile.TileContext(nc) as tc:
        build_tile_kernel(
            tc, st_d, lt_d, w_stT_d, w_ltT_d, w_gT_d, w_outT_d,
            b_st_d, b_lt_d, b_g_d, b_out_d, ln_w_d, ln_b_d, out_d,
        )
    nc.finalize()
    return nc


def build_tile_kernel(tc, st_d, lt_d, w_stT_d, w_ltT_d, w_gT_d, w_outT_d,
                      b_st_d, b_lt_d, b_g_d, b_out_d, ln_w_d, ln_b_d, out_d):
    nc = tc.nc
    from contextlib import ExitStack
    ctx = ExitStack()
    consts = ctx.enter_context(tc.tile_pool(name="consts", bufs=1))
    st_p = ctx.enter_context(tc.tile_pool(name="st", bufs=4))
    th_p = ctx.enter_context(tc.tile_pool(name="theta", bufs=1))
    lt_p = ctx.enter_context(tc.tile_pool(name="lt", bufs=2))
    phi_p = ctx.enter_context(tc.tile_pool(name="phi", bufs=2))
    g_p = ctx.enter_context(tc.tile_pool(name="g", bufs=1))
    e_p = ctx.enter_context(tc.tile_pool(name="E", bufs=1))
    tmp_p = ctx.enter_context(tc.tile_pool(name="tmp", bufs=2))
    att_p = ctx.enter_context(tc.tile_pool(name="att", bufs=1))
    y_p = ctx.enter_context(tc.tile_pool(name="y", bufs=2))
    ost_p = ctx.enter_context(tc.tile_pool(name="ost", bufs=4))
    small_p = ctx.enter_context(tc.tile_pool(name="small", bufs=2))
    ps_w = ctx.enter_context(tc.tile_pool(name="ps_work", bufs=4, space="PSUM"))
    ps_u = ctx.enter_context(tc.tile_pool(name="ps_u", bufs=4, space="PSUM"))

    # ---- constants / weights (loaded once); ordered so the critical-path
    # weights (w_stT for theta, then w_ltT/w_gT for the chunk loop) arrive
    # first and the LN/output-stage constants load in the background.
    w_stT = consts.tile([P, KT, LAT], BF)
    w_ltT = consts.tile([P, KT, LAT], BF)
    w_gT = consts.tile([P, KT, LAT], BF)
    w_outT = consts.tile([P, MT_LAT, C], BF)
    b_st = consts.tile([P, MT_LAT], F32)
    b_lt = consts.tile([P, MT_LAT], F32)
    b_g = consts.tile([P, MT_LAT], F32)
    b_out = consts.tile([P, MT_C], F32)
    ln_w = consts.tile([P, MT_LAT, S], F32)
    ln_b = consts.tile([P, MT_LAT, S], F32)
    ones_col = consts.tile([P, 1], F32)
    eps_t = consts.tile([1, 1], F32)
    # Sliced weight loads: only w_stT's first slice + b_st are issued ahead of
    # theta0; the rest are interleaved into theta0's k-loop (the Sync engine
    # issues descriptors serially, so issue order = arrival order).
    _w_stT_src = w_stT_d.ap().rearrange("(o p) m -> p o m", p=P)
    _w_ltT_src = w_ltT_d.ap().rearrange("(o p) m -> p o m", p=P)
    _w_gT_src = w_gT_d.ap().rearrange("(o p) m -> p o m", p=P)
    nc.sync.dma_start(w_stT[:, 0:4, :], _w_stT_src[:, 0:4, :])
    nc.sync.dma_start(b_st[:], b_st_d.ap().rearrange("(o p) -> p o", p=P))

    def _startup_dmas():
        for q in range(1, 4):
            yield lambda q=q: nc.sync.dma_start(
                w_stT[:, 4 * q:4 * (q + 1), :], _w_stT_src[:, 4 * q:4 * (q + 1), :])
        for q in range(4):
            yield lambda q=q: nc.sync.dma_start(
                w_ltT[:, 4 * q:4 * (q + 1), :], _w_ltT_src[:, 4 * q:4 * (q + 1), :])
            yield lambda q=q: nc.sync.dma_start(
                w_gT[:, 4 * q:4 * (q + 1), :], _w_gT_src[:, 4 * q:4 * (q + 1), :])
        yield lambda: nc.sync.dma_start(
            b_lt[:], b_lt_d.ap().rearrange("(o p) -> p o", p=P))
        yield lambda: nc.sync.dma_start(
            b_g[:], b_g_d.ap().rearrange("(o p) -> p o", p=P))
    nc.vector.memset(ones_col[:], 1.0)
    nc.vector.memset(eps_t[:], LN_EPS)

    def load_late_consts():
        nc.sync.dma_start(w_outT[:], w_outT_d.ap().rearrange("(o p) m -> p o m", p=P))
        nc.sync.dma_start(ln_w[:], ln_w_d.ap().rearrange("(o p) s -> p o s", p=P))
        nc.sync.dma_start(ln_b[:], ln_b_d.ap().rearrange("(o p) s -> p o s", p=P))
        nc.sync.dma_start(b_out[:], b_out_d.ap().rearrange("(o p) -> p o", p=P))

    # Per-sample state carried between emission stages
    state = {}

    def ensure_lt(i, cix):
        lts = state.setdefault(i, {}).setdefault("lt", {})
        if cix not in lts:
            lt_sb = lt_p.tile([P, KT, CHW], BF, tag="lt", name="lt_sb")
            nc.sync.dma_start(
                lt_sb[:],
                lt_d.ap()[i, :, cix * CHW:(cix + 1) * CHW]
                .rearrange("(o p) l -> p o l", p=P))
            lts[cix] = lt_sb
        return lts[cix]

    def emit_theta(i, extra_dmas=()):
        # st is streamed per k-tile; each tile is reused by the 4 m-tiles, so
        # the m-loop is innermost here (theta psum banks accumulate in turn).
        # extra_dmas: deferred dma_start thunks interleaved into the k-loop so
        # critical-path loads issue ahead of them on the Sync engine.
        extra = list(extra_dmas)
        st_ap = st_d.ap()[i].rearrange("(o p) s -> p o s", p=P)
        theta = th_p.tile([P, MT_LAT, S], BF, tag="theta")
        with nc.named_scope(f"theta{i}"):
            pts = [ps_w.tile([P, S], F32, tag="work", name=f"pth{m}")
                   for m in range(MT_LAT)]
            for k in range(KT):
                st_sb = st_p.tile([P, S], BF, tag="st", name="st_sb")
                nc.sync.dma_start(st_sb[:], st_ap[:, k, :])
                if k == 0 and i == 0:
                    ensure_lt(0, 0)
                if extra:
                    extra.pop(0)()
                for m in range(MT_LAT):
                    nc.tensor.matmul(pts[m][:], w_stT[:, k, m * P:(m + 1) * P],
                                     st_sb[:], start=(k == 0), stop=(k == KT - 1))
            for f in extra:
                f()
            for m in range(MT_LAT):
                nc.scalar.activation(theta[:, m, :], pts[m][:], AF.Identity,
                                     bias=b_st[:, m:m + 1])
        state.setdefault(i, {})["theta"] = theta

    def emit_chunk(i, cix):
        sti = state[i]
        theta = sti["theta"]
        if cix == 0:
            sti["g"] = g_p.tile([P, MT_L, LAT], BF, tag="g", name="g_sb")
            sti["E"] = e_p.tile([P, MT_L, S], BF, tag="E", name="e_sb")
            sti["dacc"] = tmp_p.tile([P, S], F32, tag="dacc", name="dacc")
        g_sb, e_sb, dacc = sti["g"], sti["E"], sti["dacc"]
        with nc.named_scope(f"chunks{i}"):
            if True:
                lt_sb = ensure_lt(i, cix)
                if cix + 1 < NCHUNK:
                    ensure_lt(i, cix + 1)
                # phi (LAT x CHW) for this chunk
                phi_sb = phi_p.tile([P, MT_LAT, CHW], BF, tag="phi")
                for m in range(MT_LAT):
                    pt = ps_w.tile([P, CHW], F32, tag="work")
                    for k in range(KT):
                        nc.tensor.matmul(pt[:], w_ltT[:, k, m * P:(m + 1) * P],
                                         lt_sb[:, k, :], start=(k == 0),
                                         stop=(k == KT - 1))
                    nc.vector.tensor_scalar(phi_sb[:, m, :], pt[:],
                                            b_lt[:, m:m + 1], None, ALU.add)
                # gT (CHW x LAT), 4 L-part tiles
                for j in range(MT_LAT):
                    lk = cix * MT_LAT + j
                    pt = ps_w.tile([P, LAT], F32, tag="work")
                    for k in range(KT):
                        nc.tensor.matmul(pt[:], lt_sb[:, k, j * P:(j + 1) * P],
                                         w_gT[:, k, :], start=(k == 0),
                                         stop=(k == KT - 1))
                    nc.vector.tensor_copy(g_sb[:, lk, :], pt[:])
                # scores^T (CHW x S) then E = exp(sc/sqrt(LAT))
                for j in range(MT_LAT):
                    lk = cix * MT_LAT + j
                    pt = ps_w.tile([P, S], F32, tag="work")
                    for m in range(MT_LAT):
                        nc.tensor.matmul(pt[:], phi_sb[:, m, j * P:(j + 1) * P],
                                         theta[:, m, :], start=(m == 0),
                                         stop=(m == MT_LAT - 1))
                    nc.scalar.activation(e_sb[:, lk, :], pt[:], AF.Exp,
                                         scale=INV_SQRT_LAT)
                    if lk == 0:
                        nc.vector.tensor_copy(dacc[:], e_sb[:, 0, :])
                    else:
                        nc.vector.tensor_tensor(dacc[:], dacc[:], e_sb[:, lk, :],
                                                ALU.add)

    def emit_attn(i):
        sti = state[i]
        g_sb, e_sb, dacc = sti["g"], sti["E"], sti["dacc"]
        with nc.named_scope(f"attn{i}"):
            psu = []
            for m in range(MT_LAT):
                pu = ps_u.tile([P, S], F32, tag="u")
                for lk in range(MT_L):
                    nc.tensor.matmul(pu[:], g_sb[:, lk, m * P:(m + 1) * P],
                                     e_sb[:, lk, :], start=(lk == 0),
                                     stop=(lk == MT_L - 1))
                psu.append(pu)
            # D = column sums of dacc across partitions (1 x S), fp32 matmul
            pd = ps_w.tile([1, S], F32, tag="work")
            nc.tensor.matmul(pd[:], ones_col[:], dacc[:], start=True, stop=True)
            r_sb = small_p.tile([1, S], F32, tag="r")
            nc.vector.reciprocal(r_sb[:], pd[:])
            rb = tmp_p.tile([P, S], F32, tag="rb")
            nc.gpsimd.partition_broadcast(rb[:], r_sb[:])
            att = att_p.tile([P, MT_LAT, S], F32, tag="att")
            for m in range(MT_LAT):
                nc.vector.tensor_tensor(att[:, m, :], psu[m][:], rb[:], ALU.mult)
                nc.vector.tensor_scalar(att[:, m, :], att[:, m, :],
                                        b_g[:, m:m + 1], None, ALU.add)
        sti["att"] = att

    def emit_ln(i):
        sti = state[i]
        att = sti["att"]
        with nc.named_scope(f"ln{i}"):
            # per-partition stats over the 4*S free elems
            stats = small_p.tile([P, MT_LAT, nc.vector.BN_STATS_DIM], F32, tag="bns")
            for m in range(MT_LAT):
                nc.vector.bn_stats(stats[:, m, :], att[:, m, :])
            mv = small_p.tile([P, nc.vector.BN_AGGR_DIM], F32, tag="bnm")
            nc.vector.bn_aggr(mv[:], stats[:])
            # pack [mean_p, mean_p^2 + var_p] then reduce across partitions
            t2 = small_p.tile([P, 2], F32, tag="t2")
            nc.vector.tensor_copy(t2[:, 0:1], mv[:, 0:1])
            nc.vector.tensor_tensor(t2[:, 1:2], mv[:, 0:1], mv[:, 0:1], ALU.mult)
            nc.vector.tensor_tensor(t2[:, 1:2], t2[:, 1:2], mv[:, 1:2], ALU.add)
            psm = ps_w.tile([1, 2], F32, tag="work")
            nc.tensor.matmul(psm[:], ones_col[:], t2[:], start=True, stop=True)
            sg = small_p.tile([1, 4], F32, tag="sg")
            # sg[0]=mu, sg[1]=E[x^2]
            nc.scalar.mul(sg[:, 0:2], psm[:], 1.0 / P)
            # var = E[x^2] - mu^2 -> sg[2]
            nc.vector.tensor_tensor(sg[:, 2:3], sg[:, 0:1], sg[:, 0:1], ALU.mult)
            nc.vector.tensor_tensor(sg[:, 2:3], sg[:, 1:2], sg[:, 2:3], ALU.subtract)
            # rstd = 1/sqrt(var + eps) -> sg[3]
            nc.scalar.activation(sg[:, 3:4], sg[:, 2:3], AF.Sqrt, bias=eps_t[:])
            nc.vector.reciprocal(sg[:, 3:4], sg[:, 3:4])
            musd = small_p.tile([1, 2], F32, tag="musd")
            nc.vector.tensor_copy(musd[:, 0:1], sg[:, 0:1])
            nc.vector.tensor_copy(musd[:, 1:2], sg[:, 3:4])
            musd_b = small_p.tile([P, 2], F32, tag="musdb")
            nc.gpsimd.partition_broadcast(musd_b[:], musd[:])
            y_sb = y_p.tile([P, MT_LAT, S], BF, tag="y")
            for m in range(MT_LAT):
                t = tmp_p.tile([P, S], F32, tag="lnt")
                nc.vector.tensor_scalar(t[:], att[:, m, :], musd_b[:, 0:1],
                                        musd_b[:, 1:2], ALU.subtract, ALU.mult)
                nc.vector.tensor_tensor(t[:], t[:], ln_w[:, m, :], ALU.mult)
                nc.vector.tensor_tensor(t[:], t[:], ln_b[:, m, :], ALU.add)
                nc.scalar.activation(y_sb[:, m, :], t[:], AF.Relu)
        sti["y"] = y_sb

    def emit_out(i, act_only=False):
        sti = state[i]
        y_sb = sti["y"]
        with nc.named_scope(f"out{i}"):
            out_ap = out_d.ap()[i].rearrange("(o p) s -> p o s", p=P)
            for mo in range(MT_C):
                pool = ps_w if mo % 2 == 0 else ps_u
                pt = pool.tile([P, S], F32, tag="work" if mo % 2 == 0 else "u")
                for k in range(MT_LAT):
                    nc.tensor.matmul(pt[:], w_outT[:, k, mo * P:(mo + 1) * P],
                                     y_sb[:, k, :], start=(k == 0),
                                     stop=(k == MT_LAT - 1))
                ot = ost_p.tile([P, S], F32, tag="ost")
                if act_only or mo % 2 == 0:
                    nc.scalar.activation(ot[:], pt[:], AF.Identity,
                                         bias=b_out[:, mo:mo + 1])
                else:
                    nc.vector.tensor_scalar(ot[:], pt[:], b_out[:, mo:mo + 1],
                                            None, ALU.add)
                nc.sync.dma_start(out_ap[:, mo, :], ot[:])

    # Software-pipelined emission. Sample i's LN chain (Vector-serial) is
    # emitted right after sample i+1's theta so it runs at the front of the
    # Vector queue while the PE does theta + chunk0 (~48us); the final matmuls
    # follow chunk0. Sample 0's final matmuls are held back to the tail where
    # they cover sample 3's LN chain latency.
    for i in range(NSAMP):
        emit_theta(i, extra_dmas=_startup_dmas() if i == 0 else ())
        if i > 0:
            emit_ln(i - 1)
        emit_chunk(i, 0)
        if i == 0:
            load_late_consts()
        if i > 1:
            emit_out(i - 1)
        for cix in range(1, NCHUNK):
            emit_chunk(i, cix)
        emit_attn(i)
    emit_out(0, act_only=True)
    emit_ln(NSAMP - 1)
    emit_out(NSAMP - 1)
    ctx.close()


_NC_CACHE = None


def _get_nc():
    global _NC_CACHE
    if _NC_CACHE is None:
        _NC_CACHE = build_nc()
    return _NC_CACHE


def kernel(st_feat, lt_feat, w_st, b_st, w_lt, b_lt, w_g, b_g,
           ln_w, ln_b, w_out, b_out):
    n = st_feat.shape[0]
    assert n == N_CORES * NSAMP
    bf16 = ml_dtypes.bfloat16
    st = np.asarray(st_feat, dtype=np.float32).reshape(n, C, S).astype(bf16)
    lt = np.asarray(lt_feat, dtype=np.float32).reshape(n, C, L).astype(bf16)
    w_stT = np.ascontiguousarray(np.asarray(w_st, np.float32).T).astype(bf16)
    w_ltT = np.ascontiguousarray(np.asarray(w_lt, np.float32).T).astype(bf16)
    w_gT = np.ascontiguousarray(np.asarray(w_g, np.float32).T).astype(bf16)
    w_outT = np.ascontiguousarray(np.asarray(w_out, np.float32).T).astype(bf16)
    shared = {
        "w_stT": w_stT, "w_ltT": w_ltT, "w_gT": w_gT, "w_outT": w_outT,
        "b_st": np.asarray(b_st, np.float32), "b_lt": np.asarray(b_lt, np.float32),
        "b_g": np.asarray(b_g, np.float32), "b_out": np.asarray(b_out, np.float32),
        "ln_w": np.ascontiguousarray(np.asarray(ln_w, np.float32)),
        "ln_b": np.ascontiguousarray(np.asarray(ln_b, np.float32)),
    }
    in_maps = []
    for c in range(N_CORES):
        sl = slice(c * NSAMP, (c + 1) * NSAMP)
        in_maps.append({"st": np.ascontiguousarray(st[sl]),
                        "lt": np.ascontiguousarray(lt[sl]), **shared})
    nc = _get_nc()
    res = bass_utils.run_bass_kernel_spmd(nc, in_maps, core_ids=list(range(N_CORES)))
    out = np.concatenate([res.results[c]["out"] for c in range(N_CORES)], axis=0)
    return out.reshape(n, C, S, 1, 1).astype(np.float32)


# revision 18
# speedup vs baseline: 1.1064x; 1.0161x over previous
"""Trainium2 Bass kernel for nn_NonLocalLayer (non-local attention block).

Data-parallel over batch: 32 samples -> 8 NeuronCores, 4 samples/core.
Per sample (all matmuls bf16 inputs, fp32 PSUM accumulation):
    theta = w_st @ st            (LAT=512, S=512)
    phi   = w_lt @ lt            (LAT=512, L=2048)
    gT    = (w_g @ lt)^T         (L=2048, LAT=512)   [computed transposed]
    scT   = phi^T @ theta        (L, S)              [scores transposed]
    E     = exp(scT / sqrt(LAT)) (no max-subtract; scores are O(1))
    D     = sum_L E              (1, S)
    U     = g @ E                (LAT, S)
    att   = U / D + b_g          (softmax-normalized attention output)
    LN over all (LAT, S), * ln_w + ln_b, relu
    out   = w_out @ y + b_out    (C=2048, S=512)
"""

import numpy as np
import ml_dtypes

import concourse.bacc as bacc
import concourse.mybir as mybir
import concourse.tile as tile
from concourse import bass_utils

N_CORES = 8
NSAMP = 4          # samples per core
C = 2048           # st/lt feature channels
LAT = 512          # latent channels
S = 512            # num st positions
L = 2048           # num lt positions
LN_EPS = 1e-5
P = 128
KT = C // P        # 16 contraction tiles
MT_LAT = LAT // P  # 4
MT_L = L // P      # 16
MT_C = C // P      # 16
NCHUNK = 4         # L chunks of 512
CHW = L // NCHUNK  # 512
INV_SQRT_LAT = 1.0 / float(np.sqrt(np.float32(LAT)))

BF = mybir.dt.bfloat16
F32 = mybir.dt.float32
AF = mybir.ActivationFunctionType
ALU = mybir.AluOpType


def build_nc():
    nc = bacc.Bacc("TRN2", target_bir_lowering=False, debug=False)

    st_d = nc.dram_tensor("st", (NSAMP, C, S), BF, kind="ExternalInput")
    lt_d = nc.dram_tensor("lt", (NSAMP, C, L), BF, kind="ExternalInput")
    w_stT_d = nc.dram_tensor("w_stT", (C, LAT), BF, kind="ExternalInput")
    w_ltT_d = nc.dram_tensor("w_ltT", (C, LAT), BF, kind="ExternalInput")
    w_gT_d = nc.dram_tensor("w_gT", (C, LAT), BF, kind="ExternalInput")
    w_outT_d = nc.dram_tensor("w_outT", (LAT, C), BF, kind="ExternalInput")
    b_st_d = nc.dram_tensor("b_st", (LAT,), F32, kind="ExternalInput")
    b_lt_d = nc.dram_tensor("b_lt", (LAT,), F32, kind="ExternalInput")
    b_g_d = nc.dram_tensor("b_g", (LAT,), F32, kind="ExternalInput")
    b_out_d = nc.dram_tensor("b_out", (C,), F32, kind="ExternalInput")
    ln_w_d = nc.dram_tensor("ln_w", (LAT, S), F32, kind="ExternalInput")
    ln_b_d = nc.dram_tensor("ln_b", (LAT, S), F32, kind="ExternalInput")
    out_d = nc.dram_tensor("out", (NSAMP, C, S), F32, kind="ExternalOutput")

    with tile.TileContext(nc) as tc:
        build_tile_kernel(
            tc, st_d, lt_d, w_stT_d, w_ltT_d, w_gT_d, w_outT_d,
            b_st_d, b_lt_d, b_g_d, b_out_d, ln_w_d, ln_b_d, out_d,
        )
    nc.finalize()
    return nc


def build_tile_kernel(tc, st_d, lt_d, w_stT_d, w_ltT_d, w_gT_d, w_outT_d,
                      b_st_d, b_lt_d, b_g_d, b_out_d, ln_w_d, ln_b_d, out_d):
    nc = tc.nc
    from contextlib import ExitStack
    ctx = ExitStack()
    consts = ctx.enter_context(tc.tile_pool(name="consts", bufs=1))
    st_p = ctx.enter_context(tc.tile_pool(name="st", bufs=6))
    th_p = ctx.enter_context(tc.tile_pool(name="theta", bufs=1))
    lt_p = ctx.enter_context(tc.tile_pool(name="lt", bufs=2))
    phi_p = ctx.enter_context(tc.tile_pool(name="phi", bufs=2))
    g_p = ctx.enter_context(tc.tile_pool(name="g", bufs=1))
    e_p = ctx.enter_context(tc.tile_pool(name="E", bufs=1))
    tmp_p = ctx.enter_context(tc.tile_pool(name="tmp", bufs=2))
    att_p = ctx.enter_context(tc.tile_pool(name="att", bufs=1))
    y_p = ctx.enter_context(tc.tile_pool(name="y", bufs=3))
    ost_p = ctx.enter_context(tc.tile_pool(name="ost", bufs=3))
    small_p = ctx.enter_context(tc.tile_pool(name="small", bufs=2))
    ps_w = ctx.enter_context(tc.tile_pool(name="ps_work", bufs=4, space="PSUM"))
    ps_u = ctx.enter_context(tc.tile_pool(name="ps_u", bufs=4, space="PSUM"))

    # ---- constants / weights (loaded once); ordered so the critical-path
    # weights (w_stT for theta, then w_ltT/w_gT for the chunk loop) arrive
    # first and the LN/output-stage constants load in the background.
    w_stT = consts.tile([P, KT, LAT], BF)
    w_ltT = consts.tile([P, KT, LAT], BF)
    w_gT = consts.tile([P, KT, LAT], BF)
    w_outT = consts.tile([P, MT_LAT, C], BF)
    b_st = consts.tile([P, MT_LAT], F32)
    b_lt = consts.tile([P, MT_LAT], F32)
    b_g = consts.tile([P, MT_LAT], F32)
    b_out = consts.tile([P, MT_C], F32)
    ln_w = consts.tile([P, MT_LAT, S], F32)
    ln_b = consts.tile([P, MT_LAT, S], F32)
    ones_col = consts.tile([P, 1], F32)
    eps_t = consts.tile([1, 1], F32)
    # Sliced weight loads: only w_stT's first slice + b_st are issued ahead of
    # theta0; the rest are interleaved into theta0's k-loop (the Sync engine
    # issues descriptors serially, so issue order = arrival order).
    _w_stT_src = w_stT_d.ap().rearrange("(o p) m -> p o m", p=P)
    _w_ltT_src = w_ltT_d.ap().rearrange("(o p) m -> p o m", p=P)
    _w_gT_src = w_gT_d.ap().rearrange("(o p) m -> p o m", p=P)
    nc.sync.dma_start(w_stT[:, 0:4, :], _w_stT_src[:, 0:4, :])
    nc.sync.dma_start(b_st[:], b_st_d.ap().rearrange("(o p) -> p o", p=P))

    def _startup_dmas():
        for q in range(1, 4):
            yield lambda q=q: nc.sync.dma_start(
                w_stT[:, 4 * q:4 * (q + 1), :], _w_stT_src[:, 4 * q:4 * (q + 1), :])
        yield lambda: ensure_lt(0, 0)
        for q in range(4):
            yield lambda q=q: nc.sync.dma_start(
                w_ltT[:, 4 * q:4 * (q + 1), :], _w_ltT_src[:, 4 * q:4 * (q + 1), :])
            yield lambda q=q: nc.sync.dma_start(
                w_gT[:, 4 * q:4 * (q + 1), :], _w_gT_src[:, 4 * q:4 * (q + 1), :])
        yield lambda: nc.sync.dma_start(
            b_lt[:], b_lt_d.ap().rearrange("(o p) -> p o", p=P))
        yield lambda: nc.sync.dma_start(
            b_g[:], b_g_d.ap().rearrange("(o p) -> p o", p=P))
    nc.vector.memset(ones_col[:], 1.0)
    nc.vector.memset(eps_t[:], LN_EPS)

    def load_late_consts():
        nc.sync.dma_start(w_outT[:], w_outT_d.ap().rearrange("(o p) m -> p o m", p=P))
        nc.sync.dma_start(ln_w[:], ln_w_d.ap().rearrange("(o p) s -> p o s", p=P))
        nc.sync.dma_start(ln_b[:], ln_b_d.ap().rearrange("(o p) s -> p o s", p=P))
        nc.sync.dma_start(b_out[:], b_out_d.ap().rearrange("(o p) -> p o", p=P))

    # Per-sample state carried between emission stages
    state = {}

    def ensure_lt(i, cix):
        lts = state.setdefault(i, {}).setdefault("lt", {})
        if cix not in lts:
            lt_sb = lt_p.tile([P, KT, CHW], BF, tag="lt", name="lt_sb")
            nc.sync.dma_start(
                lt_sb[:],
                lt_d.ap()[i, :, cix * CHW:(cix + 1) * CHW]
                .rearrange("(o p) l -> p o l", p=P))
            lts[cix] = lt_sb
        return lts[cix]

    def emit_theta(i, extra_dmas=()):
        # st is streamed per k-tile; each tile is reused by the 4 m-tiles, so
        # the m-loop is innermost here (theta psum banks accumulate in turn).
        # extra_dmas: deferred dma_start thunks interleaved into the k-loop so
        # critical-path loads issue ahead of them on the Sync engine.
        extra = list(extra_dmas)
        st_ap = st_d.ap()[i].rearrange("(o p) s -> p o s", p=P)
        pre = state.get(i, {}).get("st_pre", {})
        theta = th_p.tile([P, MT_LAT, S], BF, tag="theta")
        with nc.named_scope(f"theta{i}"):
            pts = [ps_w.tile([P, S], F32, tag="work", name=f"pth{m}")
                   for m in range(MT_LAT)]
            for k in range(KT):
                if k in pre:
                    st_sb = pre[k]
                else:
                    st_sb = st_p.tile([P, S], BF, tag="st", name="st_sb")
                    nc.sync.dma_start(st_sb[:], st_ap[:, k, :])
                if extra:
                    extra.pop(0)()
                for m in range(MT_LAT):
                    nc.tensor.matmul(pts[m][:], w_stT[:, k, m * P:(m + 1) * P],
                                     st_sb[:], start=(k == 0), stop=(k == KT - 1))
            for f in extra:
                f()
            for m in range(MT_LAT):
                nc.scalar.activation(theta[:, m, :], pts[m][:], AF.Identity,
                                     bias=b_st[:, m:m + 1])
        state.setdefault(i, {})["theta"] = theta

    def emit_chunk(i, cix):
        sti = state[i]
        theta = sti["theta"]
        if cix == 0:
            sti["g"] = g_p.tile([P, MT_L, LAT], BF, tag="g", name="g_sb")
            sti["E"] = e_p.tile([P, MT_L, S], BF, tag="E", name="e_sb")
            sti["dacc"] = tmp_p.tile([P, S], F32, tag="dacc", name="dacc")
        g_sb, e_sb, dacc = sti["g"], sti["E"], sti["dacc"]
        with nc.named_scope(f"chunks{i}"):
            if True:
                lt_sb = ensure_lt(i, cix)
                if cix + 1 < NCHUNK:
                    ensure_lt(i, cix + 1)
                # phi (LAT x CHW) for this chunk
                phi_sb = phi_p.tile([P, MT_LAT, CHW], BF, tag="phi")
                for m in range(MT_LAT):
                    pt = ps_w.tile([P, CHW], F32, tag="work")
                    for k in range(KT):
                        nc.tensor.matmul(pt[:], w_ltT[:, k, m * P:(m + 1) * P],
                                         lt_sb[:, k, :], start=(k == 0),
                                         stop=(k == KT - 1))
                    nc.vector.tensor_scalar(phi_sb[:, m, :], pt[:],
                                            b_lt[:, m:m + 1], None, ALU.add)
                # gT (CHW x LAT), 4 L-part tiles
                for j in range(MT_LAT):
                    lk = cix * MT_LAT + j
                    pt = ps_w.tile([P, LAT], F32, tag="work")
                    for k in range(KT):
                        nc.tensor.matmul(pt[:], lt_sb[:, k, j * P:(j + 1) * P],
                                         w_gT[:, k, :], start=(k == 0),
                                         stop=(k == KT - 1))
                    nc.vector.tensor_copy(g_sb[:, lk, :], pt[:])
                # scores^T (CHW x S) then E = exp(sc/sqrt(LAT))
                for j in range(MT_LAT):
                    lk = cix * MT_LAT + j
                    pt = ps_w.tile([P, S], F32, tag="work")
                    for m in range(MT_LAT):
                        nc.tensor.matmul(pt[:], phi_sb[:, m, j * P:(j + 1) * P],
                                         theta[:, m, :], start=(m == 0),
                                         stop=(m == MT_LAT - 1))
                    nc.scalar.activation(e_sb[:, lk, :], pt[:], AF.Exp,
                                         scale=INV_SQRT_LAT)
                    if lk == 0:
                        nc.vector.tensor_copy(dacc[:], e_sb[:, 0, :])
                    else:
                        nc.vector.tensor_tensor(dacc[:], dacc[:], e_sb[:, lk, :],
                                                ALU.add)

    def emit_attn(i):
        sti = state[i]
        g_sb, e_sb, dacc = sti["g"], sti["E"], sti["dacc"]
        with nc.named_scope(f"attn{i}"):
            psu = []
            for m in range(MT_LAT):
                pu = ps_u.tile([P, S], F32, tag="u")
                for lk in range(MT_L):
                    nc.tensor.matmul(pu[:], g_sb[:, lk, m * P:(m + 1) * P],
                                     e_sb[:, lk, :], start=(lk == 0),
                                     stop=(lk == MT_L - 1))
                psu.append(pu)
            # D = column sums of dacc across partitions (1 x S), fp32 matmul
            pd = ps_w.tile([1, S], F32, tag="work")
            nc.tensor.matmul(pd[:], ones_col[:], dacc[:], start=True, stop=True)
            r_sb = small_p.tile([1, S], F32, tag="r")
            nc.vector.reciprocal(r_sb[:], pd[:])
            rb = tmp_p.tile([P, S], F32, tag="rb")
            nc.gpsimd.partition_broadcast(rb[:], r_sb[:])
            att = att_p.tile([P, MT_LAT, S], F32, tag="att")
            for m in range(MT_LAT):
                nc.vector.tensor_tensor(att[:, m, :], psu[m][:], rb[:], ALU.mult)
                nc.vector.tensor_scalar(att[:, m, :], att[:, m, :],
                                        b_g[:, m:m + 1], None, ALU.add)
        sti["att"] = att
        if i + 1 < NSAMP:
            nxt_ap = st_d.ap()[i + 1].rearrange("(o p) s -> p o s", p=P)
            pre = state.setdefault(i + 1, {}).setdefault("st_pre", {})
            for k in range(4):
                t = st_p.tile([P, S], BF, tag="st", name="st_pre")
                nc.sync.dma_start(t[:], nxt_ap[:, k, :])
                pre[k] = t

    def emit_ln(i):
        sti = state[i]
        att = sti["att"]
        with nc.named_scope(f"ln{i}"):
            # per-partition stats over the 4*S free elems
            stats = small_p.tile([P, MT_LAT, nc.vector.BN_STATS_DIM], F32, tag="bns")
            for m in range(MT_LAT):
                nc.vector.bn_stats(stats[:, m, :], att[:, m, :])
            mv = small_p.tile([P, nc.vector.BN_AGGR_DIM], F32, tag="bnm")
            nc.vector.bn_aggr(mv[:], stats[:])
            # pack [mean_p, mean_p^2 + var_p] then reduce across partitions
            t2 = small_p.tile([P, 2], F32, tag="t2")
            nc.vector.tensor_copy(t2[:, 0:1], mv[:, 0:1])
            nc.vector.tensor_tensor(t2[:, 1:2], mv[:, 0:1], mv[:, 0:1], ALU.mult)
            nc.vector.tensor_tensor(t2[:, 1:2], t2[:, 1:2], mv[:, 1:2], ALU.add)
            psm = ps_w.tile([1, 2], F32, tag="work")
            nc.tensor.matmul(psm[:], ones_col[:], t2[:], start=True, stop=True)
            sg = small_p.tile([1, 4], F32, tag="sg")
            # sg[0]=mu, sg[1]=E[x^2]
            nc.scalar.mul(sg[:, 0:2], psm[:], 1.0 / P)
            # var = E[x^2] - mu^2 -> sg[2]
            nc.vector.tensor_tensor(sg[:, 2:3], sg[:, 0:1], sg[:, 0:1], ALU.mult)
            nc.vector.tensor_tensor(sg[:, 2:3], sg[:, 1:2], sg[:, 2:3], ALU.subtract)
            # rstd = 1/sqrt(var + eps) -> sg[3]
            nc.scalar.activation(sg[:, 3:4], sg[:, 2:3], AF.Sqrt, bias=eps_t[:])
            nc.vector.reciprocal(sg[:, 3:4], sg[:, 3:4])
            musd = small_p.tile([1, 2], F32, tag="musd")
            nc.vector.tensor_copy(musd[:, 0:1], sg[:, 0:1])
            nc.vector.tensor_copy(musd[:, 1:2], sg[:, 3:4])
            musd_b = small_p.tile([P, 2], F32, tag="musdb")
            nc.gpsimd.partition_broadcast(musd_b[:], musd[:])
            y_sb = y_p.tile([P, MT_LAT, S], BF, tag="y")
            for m in range(MT_LAT):
                t = tmp_p.tile([P, S], F32, tag="lnt")
                nc.vector.tensor_scalar(t[:], att[:, m, :], musd_b[:, 0:1],
                                        musd_b[:, 1:2], ALU.subtract, ALU.mult)
                nc.vector.tensor_tensor(t[:], t[:], ln_w[:, m, :], ALU.mult)
                nc.vector.tensor_tensor(t[:], t[:], ln_b[:, m, :], ALU.add)
                nc.scalar.activation(y_sb[:, m, :], t[:], AF.Relu)
        sti["y"] = y_sb

    def emit_out(i, act_only=False):
        sti = state[i]
        y_sb = sti["y"]
        with nc.named_scope(f"out{i}"):
            out_ap = out_d.ap()[i].rearrange("(o p) s -> p o s", p=P)
            for mo in range(MT_C):
                pool = ps_w if mo % 2 == 0 else ps_u
                pt = pool.tile([P, S], F32, tag="work" if mo % 2 == 0 else "u")
                for k in range(MT_LAT):
                    nc.tensor.matmul(pt[:], w_outT[:, k, mo * P:(mo + 1) * P],
                                     y_sb[:, k, :], start=(k == 0),
                                     stop=(k == MT_LAT - 1))
                ot = ost_p.tile([P, S], F32, tag="ost")
                if act_only or mo % 2 == 0:
                    nc.scalar.activation(ot[:], pt[:], AF.Identity,
                                         bias=b_out[:, mo:mo + 1])
                else:
                    nc.vector.tensor_scalar(ot[:], pt[:], b_out[:, mo:mo + 1],
                                            None, ALU.add)
                nc.sync.dma_start(out_ap[:, mo, :], ot[:])

    # Software-pipelined emission. Sample i's LN chain (Vector-serial) is
    # emitted right after sample i+1's theta so it runs at the front of the
    # Vector queue while the PE does theta + chunk0 (~48us); the final matmuls
    # follow chunk0. Sample 0's final matmuls are held back to the tail where
    # they cover sample 3's LN chain latency.
    for i in range(NSAMP):
        emit_theta(i, extra_dmas=_startup_dmas() if i == 0 else ())
        if i > 0:
            emit_ln(i - 1)
        emit_chunk(i, 0)
        if i == 0:
            load_late_consts()
        if i > 2:
            emit_out(i - 1)
        for cix in range(1, NCHUNK):
            emit_chunk(i, cix)
        emit_attn(i)
    emit_out(0, act_only=True)
    emit_ln(NSAMP - 1)
    emit_out(1)
    emit_out(NSAMP - 1)
    ctx.close()


_NC_CACHE = None


def _get_nc():
    global _NC_CACHE
    if _NC_CACHE is None:
        _NC_CACHE = build_nc()
    return _NC_CACHE


def kernel(st_feat, lt_feat, w_st, b_st, w_lt, b_lt, w_g, b_g,
           ln_w, ln_b, w_out, b_out):
    n = st_feat.shape[0]
    assert n == N_CORES * NSAMP
    bf16 = ml_dtypes.bfloat16
    st = np.asarray(st_feat, dtype=np.float32).reshape(n, C, S).astype(bf16)
    lt = np.asarray(lt_feat, dtype=np.float32).reshape(n, C, L).astype(bf16)
    w_stT = np.ascontiguousarray(np.asarray(w_st, np.float32).T).astype(bf16)
    w_ltT = np.ascontiguousarray(np.asarray(w_lt, np.float32).T).astype(bf16)
    w_gT = np.ascontiguousarray(np.asarray(w_g, np.float32).T).astype(bf16)
    w_outT = np.ascontiguousarray(np.asarray(w_out, np.float32).T).astype(bf16)
    shared = {
        "w_stT": w_stT, "w_ltT": w_ltT, "w_gT": w_gT, "w_outT": w_outT,
        "b_st": np.asarray(b_st, np.float32), "b_lt": np.asarray(b_lt, np.float32),
        "b_g": np.asarray(b_g, np.float32), "b_out": np.asarray(b_out, np.float32),
        "ln_w": np.ascontiguousarray(np.asarray(ln_w, np.float32)),
        "ln_b": np.ascontiguousarray(np.asarray(ln_b, np.float32)),
    }
    in_maps = []
    for c in range(N_CORES):
        sl = slice(c * NSAMP, (c + 1) * NSAMP)
        in_maps.append({"st": np.ascontiguousarray(st[sl]),
                        "lt": np.ascontiguousarray(lt[sl]), **shared})
    nc = _get_nc()
    res = bass_utils.run_bass_kernel_spmd(nc, in_maps, core_ids=list(range(N_CORES)))
    out = np.concatenate([res.results[c]["out"] for c in range(N_CORES)], axis=0)
    return out.reshape(n, C, S, 1, 1).astype(np.float32)


# revision 20
# speedup vs baseline: 1.1190x; 1.0114x over previous
"""Trainium2 Bass kernel for nn_NonLocalLayer (non-local attention block).

Data-parallel over batch: 32 samples -> 8 NeuronCores, 4 samples/core.
Per sample (all matmuls bf16 inputs, fp32 PSUM accumulation):
    theta = w_st @ st            (LAT=512, S=512)
    phi   = w_lt @ lt            (LAT=512, L=2048)
    gT    = (w_g @ lt)^T         (L=2048, LAT=512)   [computed transposed]
    scT   = phi^T @ theta        (L, S)              [scores transposed]
    E     = exp(scT / sqrt(LAT)) (no max-subtract; scores are O(1))
    D     = sum_L E              (1, S)
    U     = g @ E                (LAT, S)
    att   = U / D + b_g          (softmax-normalized attention output)
    LN over all (LAT, S), * ln_w + ln_b, relu
    out   = w_out @ y + b_out    (C=2048, S=512)
"""

import numpy as np
import ml_dtypes

import concourse.bacc as bacc
import concourse.mybir as mybir
import concourse.tile as tile
from concourse import bass_utils

N_CORES = 8
NSAMP = 4          # samples per core
C = 2048           # st/lt feature channels
LAT = 512          # latent channels
S = 512            # num st positions
L = 2048           # num lt positions
LN_EPS = 1e-5
P = 128
KT = C // P        # 16 contraction tiles
MT_LAT = LAT // P  # 4
MT_L = L // P      # 16
MT_C = C // P      # 16
NCHUNK = 4         # L chunks of 512
CHW = L // NCHUNK  # 512
INV_SQRT_LAT = 1.0 / float(np.sqrt(np.float32(LAT)))

BF = mybir.dt.bfloat16
F32 = mybir.dt.float32
AF = mybir.ActivationFunctionType
ALU = mybir.AluOpType


def build_nc():
    nc = bacc.Bacc("TRN2", target_bir_lowering=False, debug=False)

    st_d = nc.dram_tensor("st", (NSAMP, C, S), BF, kind="ExternalInput")
    lt_d = nc.dram_tensor("lt", (NSAMP, C, L), BF, kind="ExternalInput")
    w_stT_d = nc.dram_tensor("w_stT", (C, LAT), BF, kind="ExternalInput")
    w_ltT_d = nc.dram_tensor("w_ltT", (C, LAT), BF, kind="ExternalInput")
    w_gT_d = nc.dram_tensor("w_gT", (C, LAT), BF, kind="ExternalInput")
    w_outT_d = nc.dram_tensor("w_outT", (LAT, C), BF, kind="ExternalInput")
    b_st_d = nc.dram_tensor("b_st", (LAT,), F32, kind="ExternalInput")
    b_lt_d = nc.dram_tensor("b_lt", (LAT,), F32, kind="ExternalInput")
    b_g_d = nc.dram_tensor("b_g", (LAT,), F32, kind="ExternalInput")
    b_out_d = nc.dram_tensor("b_out", (C,), F32, kind="ExternalInput")
    ln_w_d = nc.dram_tensor("ln_w", (LAT, S), F32, kind="ExternalInput")
    ln_b_d = nc.dram_tensor("ln_b", (LAT, S), F32, kind="ExternalInput")
    out_d = nc.dram_tensor("out", (NSAMP, C, S), F32, kind="ExternalOutput")

    with tile.TileContext(nc) as tc:
        build_tile_kernel(
            tc, st_d, lt_d, w_stT_d, w_ltT_d, w_gT_d, w_outT_d,
            b_st_d, b_lt_d, b_g_d, b_out_d, ln_w_d, ln_b_d, out_d,
        )
    nc.finalize()
    return nc


def build_tile_kernel(tc, st_d, lt_d, w_stT_d, w_ltT_d, w_gT_d, w_outT_d,
                      b_st_d, b_lt_d, b_g_d, b_out_d, ln_w_d, ln_b_d, out_d):
    nc = tc.nc
    from contextlib import ExitStack
    ctx = ExitStack()
    consts = ctx.enter_context(tc.tile_pool(name="consts", bufs=1))
    st_p = ctx.enter_context(tc.tile_pool(name="st", bufs=6))
    th_p = ctx.enter_context(tc.tile_pool(name="theta", bufs=1))
    lt_p = ctx.enter_context(tc.tile_pool(name="lt", bufs=2))
    phi_p = ctx.enter_context(tc.tile_pool(name="phi", bufs=2))
    g_p = ctx.enter_context(tc.tile_pool(name="g", bufs=1))
    e_p = ctx.enter_context(tc.tile_pool(name="E", bufs=1))
    tmp_p = ctx.enter_context(tc.tile_pool(name="tmp", bufs=2))
    att_p = ctx.enter_context(tc.tile_pool(name="att", bufs=1))
    y_p = ctx.enter_context(tc.tile_pool(name="y", bufs=3))
    ost_p = ctx.enter_context(tc.tile_pool(name="ost", bufs=3))
    small_p = ctx.enter_context(tc.tile_pool(name="small", bufs=2))
    ps_w = ctx.enter_context(tc.tile_pool(name="ps_work", bufs=4, space="PSUM"))
    ps_u = ctx.enter_context(tc.tile_pool(name="ps_u", bufs=4, space="PSUM"))

    # ---- constants / weights (loaded once); ordered so the critical-path
    # weights (w_stT for theta, then w_ltT/w_gT for the chunk loop) arrive
    # first and the LN/output-stage constants load in the background.
    w_stT = consts.tile([P, KT, LAT], BF)
    w_ltT = consts.tile([P, KT, LAT], BF)
    w_gT = consts.tile([P, KT, LAT], BF)
    w_outT = consts.tile([P, MT_LAT, C], BF)
    b_st = consts.tile([P, MT_LAT], F32)
    b_lt = consts.tile([P, MT_LAT], F32)
    b_g = consts.tile([P, MT_LAT], F32)
    b_out = consts.tile([P, MT_C], F32)
    ln_w = consts.tile([P, MT_LAT, S], F32)
    ln_b = consts.tile([P, MT_LAT, S], F32)
    eps_t = consts.tile([P, 1], F32)
    # Sliced weight loads: only w_stT's first slice + b_st are issued ahead of
    # theta0; the rest are interleaved into theta0's k-loop (the Sync engine
    # issues descriptors serially, so issue order = arrival order).
    _w_stT_src = w_stT_d.ap().rearrange("(o p) m -> p o m", p=P)
    _w_ltT_src = w_ltT_d.ap().rearrange("(o p) m -> p o m", p=P)
    _w_gT_src = w_gT_d.ap().rearrange("(o p) m -> p o m", p=P)
    nc.sync.dma_start(w_stT[:, 0:4, :], _w_stT_src[:, 0:4, :])
    nc.sync.dma_start(b_st[:], b_st_d.ap().rearrange("(o p) -> p o", p=P))

    def _startup_dmas():
        for q in range(1, 4):
            yield lambda q=q: nc.sync.dma_start(
                w_stT[:, 4 * q:4 * (q + 1), :], _w_stT_src[:, 4 * q:4 * (q + 1), :])
        yield lambda: ensure_lt(0, 0)
        for q in range(4):
            yield lambda q=q: nc.sync.dma_start(
                w_ltT[:, 4 * q:4 * (q + 1), :], _w_ltT_src[:, 4 * q:4 * (q + 1), :])
            yield lambda q=q: nc.sync.dma_start(
                w_gT[:, 4 * q:4 * (q + 1), :], _w_gT_src[:, 4 * q:4 * (q + 1), :])
        yield lambda: nc.sync.dma_start(
            b_lt[:], b_lt_d.ap().rearrange("(o p) -> p o", p=P))
        yield lambda: nc.sync.dma_start(
            b_g[:], b_g_d.ap().rearrange("(o p) -> p o", p=P))
    nc.vector.memset(eps_t[:], LN_EPS)

    def load_late_consts():
        nc.sync.dma_start(w_outT[:], w_outT_d.ap().rearrange("(o p) m -> p o m", p=P))
        nc.sync.dma_start(ln_w[:], ln_w_d.ap().rearrange("(o p) s -> p o s", p=P))
        nc.sync.dma_start(ln_b[:], ln_b_d.ap().rearrange("(o p) s -> p o s", p=P))
        nc.sync.dma_start(b_out[:], b_out_d.ap().rearrange("(o p) -> p o", p=P))

    # Per-sample state carried between emission stages
    state = {}

    def ensure_lt(i, cix):
        lts = state.setdefault(i, {}).setdefault("lt", {})
        if cix not in lts:
            lt_sb = lt_p.tile([P, KT, CHW], BF, tag="lt", name="lt_sb")
            nc.sync.dma_start(
                lt_sb[:],
                lt_d.ap()[i, :, cix * CHW:(cix + 1) * CHW]
                .rearrange("(o p) l -> p o l", p=P))
            lts[cix] = lt_sb
        return lts[cix]

    def emit_theta(i, extra_dmas=()):
        # st is streamed per k-tile; each tile is reused by the 4 m-tiles, so
        # the m-loop is innermost here (theta psum banks accumulate in turn).
        # extra_dmas: deferred dma_start thunks interleaved into the k-loop so
        # critical-path loads issue ahead of them on the Sync engine.
        extra = list(extra_dmas)
        st_ap = st_d.ap()[i].rearrange("(o p) s -> p o s", p=P)
        pre = state.get(i, {}).get("st_pre", {})
        theta = th_p.tile([P, MT_LAT, S], BF, tag="theta")
        with nc.named_scope(f"theta{i}"):
            pts = [ps_w.tile([P, S], F32, tag="work", name=f"pth{m}")
                   for m in range(MT_LAT)]
            for k in range(KT):
                if k in pre:
                    st_sb = pre[k]
                else:
                    st_sb = st_p.tile([P, S], BF, tag="st", name="st_sb")
                    nc.sync.dma_start(st_sb[:], st_ap[:, k, :])
                if extra:
                    extra.pop(0)()
                for m in range(MT_LAT):
                    nc.tensor.matmul(pts[m][:], w_stT[:, k, m * P:(m + 1) * P],
                                     st_sb[:], start=(k == 0), stop=(k == KT - 1))
            for f in extra:
                f()
            for m in range(MT_LAT):
                nc.scalar.activation(theta[:, m, :], pts[m][:], AF.Identity,
                                     bias=b_st[:, m:m + 1])
        state.setdefault(i, {})["theta"] = theta

    def emit_chunk(i, cix):
        sti = state[i]
        theta = sti["theta"]
        if cix == 0:
            sti["g"] = g_p.tile([P, MT_L, LAT], BF, tag="g", name="g_sb")
            sti["E"] = e_p.tile([P, MT_L, S], BF, tag="E", name="e_sb")
            sti["dacc"] = tmp_p.tile([P, S], F32, tag="dacc", name="dacc")
        g_sb, e_sb, dacc = sti["g"], sti["E"], sti["dacc"]
        with nc.named_scope(f"chunks{i}"):
            if True:
                lt_sb = ensure_lt(i, cix)
                if cix + 1 < NCHUNK:
                    ensure_lt(i, cix + 1)
                # phi (LAT x CHW) for this chunk
                phi_sb = phi_p.tile([P, MT_LAT, CHW], BF, tag="phi")
                for m in range(MT_LAT):
                    pt = ps_w.tile([P, CHW], F32, tag="work")
                    for k in range(KT):
                        nc.tensor.matmul(pt[:], w_ltT[:, k, m * P:(m + 1) * P],
                                         lt_sb[:, k, :], start=(k == 0),
                                         stop=(k == KT - 1))
                    nc.vector.tensor_scalar(phi_sb[:, m, :], pt[:],
                                            b_lt[:, m:m + 1], None, ALU.add)
                # gT (CHW x LAT), 4 L-part tiles
                for j in range(MT_LAT):
                    lk = cix * MT_LAT + j
                    pt = ps_w.tile([P, LAT], F32, tag="work")
                    for k in range(KT):
                        nc.tensor.matmul(pt[:], lt_sb[:, k, j * P:(j + 1) * P],
                                         w_gT[:, k, :], start=(k == 0),
                                         stop=(k == KT - 1))
                    nc.vector.tensor_copy(g_sb[:, lk, :], pt[:])
                # scores^T (CHW x S) then E = exp(sc/sqrt(LAT))
                for j in range(MT_LAT):
                    lk = cix * MT_LAT + j
                    pt = ps_w.tile([P, S], F32, tag="work")
                    for m in range(MT_LAT):
                        nc.tensor.matmul(pt[:], phi_sb[:, m, j * P:(j + 1) * P],
                                         theta[:, m, :], start=(m == 0),
                                         stop=(m == MT_LAT - 1))
                    nc.scalar.activation(e_sb[:, lk, :], pt[:], AF.Exp,
                                         scale=INV_SQRT_LAT)
                    if lk == 0:
                        nc.vector.tensor_copy(dacc[:], e_sb[:, 0, :])
                    else:
                        nc.vector.tensor_tensor(dacc[:], dacc[:], e_sb[:, lk, :],
                                                ALU.add)

    def emit_attn(i):
        sti = state[i]
        g_sb, e_sb = sti["g"], sti["E"]
        with nc.named_scope(f"attn{i}"):
            psu = []
            for m in range(MT_LAT):
                pu = ps_u.tile([P, S], F32, tag="u")
                for lk in range(MT_L):
                    nc.tensor.matmul(pu[:], g_sb[:, lk, m * P:(m + 1) * P],
                                     e_sb[:, lk, :], start=(lk == 0),
                                     stop=(lk == MT_L - 1))
                psu.append(pu)
        sti["psu"] = psu
        if i + 1 < NSAMP:
            nxt_ap = st_d.ap()[i + 1].rearrange("(o p) s -> p o s", p=P)
            pre = state.setdefault(i + 1, {}).setdefault("st_pre", {})
            for k in range(4):
                t = st_p.tile([P, S], BF, tag="st", name="st_pre")
                nc.sync.dma_start(t[:], nxt_ap[:, k, :])
                pre[k] = t

    def emit_norm(i):
        sti = state[i]
        dacc, psu = sti["dacc"], sti["psu"]
        with nc.named_scope(f"attn{i}"):
            # D = column sums of dacc across partitions, computed on GpSimd so
            # the PE never blocks on the softmax denominator.
            from concourse import bass_isa
            dall = tmp_p.tile([P, S], F32, tag="dall")
            nc.gpsimd.partition_all_reduce(dall[:], dacc[:], P, bass_isa.ReduceOp.add)
            rb = tmp_p.tile([P, S], F32, tag="rb")
            nc.vector.reciprocal(rb[:], dall[:])
            att = att_p.tile([P, MT_LAT, S], F32, tag="att")
            for m in range(MT_LAT):
                nc.vector.tensor_tensor(att[:, m, :], psu[m][:], rb[:], ALU.mult)
                nc.vector.tensor_scalar(att[:, m, :], att[:, m, :],
                                        b_g[:, m:m + 1], None, ALU.add)
        sti["att"] = att

    def emit_ln(i):
        sti = state[i]
        att = sti["att"]
        with nc.named_scope(f"ln{i}"):
            # per-partition stats over the 4*S free elems
            stats = small_p.tile([P, MT_LAT, nc.vector.BN_STATS_DIM], F32, tag="bns")
            for m in range(MT_LAT):
                nc.vector.bn_stats(stats[:, m, :], att[:, m, :])
            mv = small_p.tile([P, nc.vector.BN_AGGR_DIM], F32, tag="bnm")
            nc.vector.bn_aggr(mv[:], stats[:])
            # pack [mean_p, mean_p^2 + var_p] then reduce across partitions
            t2 = small_p.tile([P, 2], F32, tag="t2")
            nc.vector.tensor_copy(t2[:, 0:1], mv[:, 0:1])
            nc.vector.tensor_tensor(t2[:, 1:2], mv[:, 0:1], mv[:, 0:1], ALU.mult)
            nc.vector.tensor_tensor(t2[:, 1:2], t2[:, 1:2], mv[:, 1:2], ALU.add)
            from concourse import bass_isa
            t2r = small_p.tile([P, 2], F32, tag="t2r")
            nc.gpsimd.partition_all_reduce(t2r[:], t2[:], P, bass_isa.ReduceOp.add)
            sg = small_p.tile([P, 4], F32, tag="sg")
            # sg[:,0]=mu, sg[:,1]=E[x^2] (same value on every partition)
            nc.scalar.mul(sg[:, 0:2], t2r[:], 1.0 / P)
            # var = E[x^2] - mu^2 -> sg[:,2]
            nc.vector.tensor_tensor(sg[:, 2:3], sg[:, 0:1], sg[:, 0:1], ALU.mult)
            nc.vector.tensor_tensor(sg[:, 2:3], sg[:, 1:2], sg[:, 2:3], ALU.subtract)
            # rstd = 1/sqrt(var + eps) -> sg[:,3]
            nc.scalar.activation(sg[:, 3:4], sg[:, 2:3], AF.Sqrt, bias=eps_t[:])
            nc.vector.reciprocal(sg[:, 3:4], sg[:, 3:4])
            musd_b = sg
            y_sb = y_p.tile([P, MT_LAT, S], BF, tag="y")
            for m in range(MT_LAT):
                t = tmp_p.tile([P, S], F32, tag="lnt")
                nc.vector.tensor_scalar(t[:], att[:, m, :], musd_b[:, 0:1],
                                        musd_b[:, 3:4], ALU.subtract, ALU.mult)
                nc.vector.tensor_tensor(t[:], t[:], ln_w[:, m, :], ALU.mult)
                nc.vector.tensor_tensor(t[:], t[:], ln_b[:, m, :], ALU.add)
                nc.scalar.activation(y_sb[:, m, :], t[:], AF.Relu)
        sti["y"] = y_sb

    def emit_out(i, act_only=False):
        sti = state[i]
        y_sb = sti["y"]
        with nc.named_scope(f"out{i}"):
            out_ap = out_d.ap()[i].rearrange("(o p) s -> p o s", p=P)
            for mo in range(MT_C):
                pool = ps_w if mo % 2 == 0 else ps_u
                pt = pool.tile([P, S], F32, tag="work" if mo % 2 == 0 else "u")
                for k in range(MT_LAT):
                    nc.tensor.matmul(pt[:], w_outT[:, k, mo * P:(mo + 1) * P],
                                     y_sb[:, k, :], start=(k == 0),
                                     stop=(k == MT_LAT - 1))
                ot = ost_p.tile([P, S], F32, tag="ost")
                if act_only or mo % 2 == 0:
                    nc.scalar.activation(ot[:], pt[:], AF.Identity,
                                         bias=b_out[:, mo:mo + 1])
                else:
                    nc.vector.tensor_scalar(ot[:], pt[:], b_out[:, mo:mo + 1],
                                            None, ALU.add)
                nc.sync.dma_start(out_ap[:, mo, :], ot[:])

    # Software-pipelined emission. Sample i's LN chain (Vector-serial) is
    # emitted right after sample i+1's theta so it runs at the front of the
    # Vector queue while the PE does theta + chunk0 (~48us); the final matmuls
    # follow chunk0. Sample 0's final matmuls are held back to the tail where
    # they cover sample 3's LN chain latency.
    for i in range(NSAMP):
        emit_theta(i, extra_dmas=_startup_dmas() if i == 0 else ())
        if i > 0:
            emit_norm(i - 1)
            emit_ln(i - 1)
        emit_chunk(i, 0)
        if i == 0:
            load_late_consts()
        if i > 2:
            emit_out(i - 1)
        for cix in range(1, NCHUNK):
            emit_chunk(i, cix)
        emit_attn(i)
    emit_norm(NSAMP - 1)
    emit_out(0, act_only=True)
    emit_ln(NSAMP - 1)
    emit_out(1)
    emit_out(NSAMP - 1)
    ctx.close()


_NC_CACHE = None


def _get_nc():
    global _NC_CACHE
    if _NC_CACHE is None:
        _NC_CACHE = build_nc()
    return _NC_CACHE


def kernel(st_feat, lt_feat, w_st, b_st, w_lt, b_lt, w_g, b_g,
           ln_w, ln_b, w_out, b_out):
    n = st_feat.shape[0]
    assert n == N_CORES * NSAMP
    bf16 = ml_dtypes.bfloat16
    st = np.asarray(st_feat, dtype=np.float32).reshape(n, C, S).astype(bf16)
    lt = np.asarray(lt_feat, dtype=np.float32).reshape(n, C, L).astype(bf16)
    w_stT = np.ascontiguousarray(np.asarray(w_st, np.float32).T).astype(bf16)
    w_ltT = np.ascontiguousarray(np.asarray(w_lt, np.float32).T).astype(bf16)
    w_gT = np.ascontiguousarray(np.asarray(w_g, np.float32).T).astype(bf16)
    w_outT = np.ascontiguousarray(np.asarray(w_out, np.float32).T).astype(bf16)
    shared = {
        "w_stT": w_stT, "w_ltT": w_ltT, "w_gT": w_gT, "w_outT": w_outT,
        "b_st": np.asarray(b_st, np.float32), "b_lt": np.asarray(b_lt, np.float32),
        "b_g": np.asarray(b_g, np.float32), "b_out": np.asarray(b_out, np.float32),
        "ln_w": np.ascontiguousarray(np.asarray(ln_w, np.float32)),
        "ln_b": np.ascontiguousarray(np.asarray(ln_b, np.float32)),
    }
    in_maps = []
    for c in range(N_CORES):
        sl = slice(c * NSAMP, (c + 1) * NSAMP)
        in_maps.append({"st": np.ascontiguousarray(st[sl]),
                        "lt": np.ascontiguousarray(lt[sl]), **shared})
    nc = _get_nc()
    res = bass_utils.run_bass_kernel_spmd(nc, in_maps, core_ids=list(range(N_CORES)))
    out = np.concatenate([res.results[c]["out"] for c in range(N_CORES)], axis=0)
    return out.reshape(n, C, S, 1, 1).astype(np.float32)


# revision 21
# speedup vs baseline: 1.1355x; 1.0148x over previous
"""Trainium2 Bass kernel for nn_NonLocalLayer (non-local attention block).

Data-parallel over batch: 32 samples -> 8 NeuronCores, 4 samples/core.
Per sample (all matmuls bf16 inputs, fp32 PSUM accumulation):
    theta = w_st @ st            (LAT=512, S=512)
    phi   = w_lt @ lt            (LAT=512, L=2048)
    gT    = (w_g @ lt)^T         (L=2048, LAT=512)   [computed transposed]
    scT   = phi^T @ theta        (L, S)              [scores transposed]
    E     = exp(scT / sqrt(LAT)) (no max-subtract; scores are O(1))
    D     = sum_L E              (1, S)
    U     = g @ E                (LAT, S)
    att   = U / D + b_g          (softmax-normalized attention output)
    LN over all (LAT, S), * ln_w + ln_b, relu
    out   = w_out @ y + b_out    (C=2048, S=512)
"""

import numpy as np
import ml_dtypes

import concourse.bacc as bacc
import concourse.mybir as mybir
import concourse.tile as tile
from concourse import bass_utils

N_CORES = 8
NSAMP = 4          # samples per core
C = 2048           # st/lt feature channels
LAT = 512          # latent channels
S = 512            # num st positions
L = 2048           # num lt positions
LN_EPS = 1e-5
P = 128
KT = C // P        # 16 contraction tiles
MT_LAT = LAT // P  # 4
MT_L = L // P      # 16
MT_C = C // P      # 16
NCHUNK = 4         # L chunks of 512
CHW = L // NCHUNK  # 512
INV_SQRT_LAT = 1.0 / float(np.sqrt(np.float32(LAT)))

BF = mybir.dt.bfloat16
F32 = mybir.dt.float32
AF = mybir.ActivationFunctionType
ALU = mybir.AluOpType


def build_nc():
    nc = bacc.Bacc("TRN2", target_bir_lowering=False, debug=False)

    st_d = nc.dram_tensor("st", (NSAMP, C, S), BF, kind="ExternalInput")
    lt_d = nc.dram_tensor("lt", (NSAMP, C, L), BF, kind="ExternalInput")
    w_stT_d = nc.dram_tensor("w_stT", (C, LAT), BF, kind="ExternalInput")
    w_ltT_d = nc.dram_tensor("w_ltT", (C, LAT), BF, kind="ExternalInput")
    w_gT_d = nc.dram_tensor("w_gT", (C, LAT), BF, kind="ExternalInput")
    w_outT_d = nc.dram_tensor("w_outT", (LAT, C), BF, kind="ExternalInput")
    b_st_d = nc.dram_tensor("b_st", (LAT,), F32, kind="ExternalInput")
    b_lt_d = nc.dram_tensor("b_lt", (LAT,), F32, kind="ExternalInput")
    b_g_d = nc.dram_tensor("b_g", (LAT,), F32, kind="ExternalInput")
    b_out_d = nc.dram_tensor("b_out", (C,), F32, kind="ExternalInput")
    ln_w_d = nc.dram_tensor("ln_w", (LAT, S), F32, kind="ExternalInput")
    ln_b_d = nc.dram_tensor("ln_b", (LAT, S), F32, kind="ExternalInput")
    out_d = nc.dram_tensor("out", (NSAMP, C, S), F32, kind="ExternalOutput")

    with tile.TileContext(nc) as tc:
        build_tile_kernel(
            tc, st_d, lt_d, w_stT_d, w_ltT_d, w_gT_d, w_outT_d,
            b_st_d, b_lt_d, b_g_d, b_out_d, ln_w_d, ln_b_d, out_d,
        )
    nc.finalize()
    return nc


def build_tile_kernel(tc, st_d, lt_d, w_stT_d, w_ltT_d, w_gT_d, w_outT_d,
                      b_st_d, b_lt_d, b_g_d, b_out_d, ln_w_d, ln_b_d, out_d):
    nc = tc.nc
    from contextlib import ExitStack
    ctx = ExitStack()
    consts = ctx.enter_context(tc.tile_pool(name="consts", bufs=1))
    st_p = ctx.enter_context(tc.tile_pool(name="st", bufs=6))
    th_p = ctx.enter_context(tc.tile_pool(name="theta", bufs=1))
    lt_p = ctx.enter_context(tc.tile_pool(name="lt", bufs=2))
    phi_p = ctx.enter_context(tc.tile_pool(name="phi", bufs=2))
    g_p = ctx.enter_context(tc.tile_pool(name="g", bufs=1))
    e_p = ctx.enter_context(tc.tile_pool(name="E", bufs=1))
    tmp_p = ctx.enter_context(tc.tile_pool(name="tmp", bufs=2))
    att_p = ctx.enter_context(tc.tile_pool(name="att", bufs=1))
    y_p = ctx.enter_context(tc.tile_pool(name="y", bufs=3))
    ost_p = ctx.enter_context(tc.tile_pool(name="ost", bufs=3))
    small_p = ctx.enter_context(tc.tile_pool(name="small", bufs=2))
    ps_w = ctx.enter_context(tc.tile_pool(name="ps_work", bufs=4, space="PSUM"))
    ps_u = ctx.enter_context(tc.tile_pool(name="ps_u", bufs=4, space="PSUM"))

    # ---- constants / weights (loaded once); ordered so the critical-path
    # weights (w_stT for theta, then w_ltT/w_gT for the chunk loop) arrive
    # first and the LN/output-stage constants load in the background.
    w_stT = consts.tile([P, KT, LAT], BF)
    w_ltT = consts.tile([P, KT, LAT], BF)
    w_gT = consts.tile([P, KT, LAT], BF)
    w_outT = consts.tile([P, MT_LAT, C], BF)
    b_st = consts.tile([P, MT_LAT], F32)
    b_lt = consts.tile([P, MT_LAT], F32)
    b_g = consts.tile([P, MT_LAT], F32)
    b_out = consts.tile([P, MT_C], F32)
    ln_w = consts.tile([P, MT_LAT, S], F32)
    ln_b = consts.tile([P, MT_LAT, S], F32)
    eps_t = consts.tile([P, 1], F32)
    # Sliced weight loads: only w_stT's first slice + b_st are issued ahead of
    # theta0; the rest are interleaved into theta0's k-loop (the Sync engine
    # issues descriptors serially, so issue order = arrival order).
    _w_stT_src = w_stT_d.ap().rearrange("(o p) m -> p o m", p=P)
    _w_ltT_src = w_ltT_d.ap().rearrange("(o p) m -> p o m", p=P)
    _w_gT_src = w_gT_d.ap().rearrange("(o p) m -> p o m", p=P)
    nc.sync.dma_start(w_stT[:, 0:2, :], _w_stT_src[:, 0:2, :])
    nc.sync.dma_start(b_st[:], b_st_d.ap().rearrange("(o p) -> p o", p=P))

    def _startup_dmas():
        yield lambda: nc.sync.dma_start(w_stT[:, 2:4, :], _w_stT_src[:, 2:4, :])
        for q in range(1, 4):
            yield lambda q=q: nc.sync.dma_start(
                w_stT[:, 4 * q:4 * (q + 1), :], _w_stT_src[:, 4 * q:4 * (q + 1), :])
        yield lambda: ensure_lt(0, 0)
        for q in range(4):
            yield lambda q=q: nc.sync.dma_start(
                w_ltT[:, 4 * q:4 * (q + 1), :], _w_ltT_src[:, 4 * q:4 * (q + 1), :])
            yield lambda q=q: nc.sync.dma_start(
                w_gT[:, 4 * q:4 * (q + 1), :], _w_gT_src[:, 4 * q:4 * (q + 1), :])
        yield lambda: nc.sync.dma_start(
            b_lt[:], b_lt_d.ap().rearrange("(o p) -> p o", p=P))
        yield lambda: nc.sync.dma_start(
            b_g[:], b_g_d.ap().rearrange("(o p) -> p o", p=P))
    nc.vector.memset(eps_t[:], LN_EPS)

    def load_late_consts():
        nc.sync.dma_start(w_outT[:], w_outT_d.ap().rearrange("(o p) m -> p o m", p=P))
        nc.sync.dma_start(ln_w[:], ln_w_d.ap().rearrange("(o p) s -> p o s", p=P))
        nc.sync.dma_start(ln_b[:], ln_b_d.ap().rearrange("(o p) s -> p o s", p=P))
        nc.sync.dma_start(b_out[:], b_out_d.ap().rearrange("(o p) -> p o", p=P))

    # Per-sample state carried between emission stages
    state = {}

    def ensure_lt(i, cix):
        lts = state.setdefault(i, {}).setdefault("lt", {})
        if cix not in lts:
            lt_sb = lt_p.tile([P, KT, CHW], BF, tag="lt", name="lt_sb")
            nc.sync.dma_start(
                lt_sb[:],
                lt_d.ap()[i, :, cix * CHW:(cix + 1) * CHW]
                .rearrange("(o p) l -> p o l", p=P))
            lts[cix] = lt_sb
        return lts[cix]

    def emit_theta(i, extra_dmas=()):
        # st is streamed per k-tile; each tile is reused by the 4 m-tiles, so
        # the m-loop is innermost here (theta psum banks accumulate in turn).
        # extra_dmas: deferred dma_start thunks interleaved into the k-loop so
        # critical-path loads issue ahead of them on the Sync engine.
        extra = list(extra_dmas)
        st_ap = st_d.ap()[i].rearrange("(o p) s -> p o s", p=P)
        pre = state.get(i, {}).get("st_pre", {})
        theta = th_p.tile([P, MT_LAT, S], BF, tag="theta")
        with nc.named_scope(f"theta{i}"):
            pts = [ps_w.tile([P, S], F32, tag="work", name=f"pth{m}")
                   for m in range(MT_LAT)]
            for k in range(KT):
                if k in pre:
                    st_sb = pre[k]
                else:
                    st_sb = st_p.tile([P, S], BF, tag="st", name="st_sb")
                    nc.sync.dma_start(st_sb[:], st_ap[:, k, :])
                if extra:
                    extra.pop(0)()
                for m in range(MT_LAT):
                    nc.tensor.matmul(pts[m][:], w_stT[:, k, m * P:(m + 1) * P],
                                     st_sb[:], start=(k == 0), stop=(k == KT - 1))
            for f in extra:
                f()
            for m in range(MT_LAT):
                nc.scalar.activation(theta[:, m, :], pts[m][:], AF.Identity,
                                     bias=b_st[:, m:m + 1])
        state.setdefault(i, {})["theta"] = theta

    def emit_chunk(i, cix):
        sti = state[i]
        theta = sti["theta"]
        if cix == 0:
            sti["g"] = g_p.tile([P, MT_L, LAT], BF, tag="g", name="g_sb")
            sti["E"] = e_p.tile([P, MT_L, S], BF, tag="E", name="e_sb")
            sti["dacc"] = tmp_p.tile([P, S], F32, tag="dacc", name="dacc")
        g_sb, e_sb, dacc = sti["g"], sti["E"], sti["dacc"]
        with nc.named_scope(f"chunks{i}"):
            if True:
                lt_sb = ensure_lt(i, cix)
                if cix + 1 < NCHUNK:
                    ensure_lt(i, cix + 1)
                # phi (LAT x CHW) for this chunk
                phi_sb = phi_p.tile([P, MT_LAT, CHW], BF, tag="phi")
                for m in range(MT_LAT):
                    pt = ps_w.tile([P, CHW], F32, tag="work")
                    for k in range(KT):
                        nc.tensor.matmul(pt[:], w_ltT[:, k, m * P:(m + 1) * P],
                                         lt_sb[:, k, :], start=(k == 0),
                                         stop=(k == KT - 1))
                    nc.vector.tensor_scalar(phi_sb[:, m, :], pt[:],
                                            b_lt[:, m:m + 1], None, ALU.add)
                # gT (CHW x LAT), 4 L-part tiles
                for j in range(MT_LAT):
                    lk = cix * MT_LAT + j
                    pt = ps_w.tile([P, LAT], F32, tag="work")
                    for k in range(KT):
                        nc.tensor.matmul(pt[:], lt_sb[:, k, j * P:(j + 1) * P],
                                         w_gT[:, k, :], start=(k == 0),
                                         stop=(k == KT - 1))
                    nc.vector.tensor_copy(g_sb[:, lk, :], pt[:])
                # scores^T (CHW x S) then E = exp(sc/sqrt(LAT))
                for j in range(MT_LAT):
                    lk = cix * MT_LAT + j
                    pt = ps_w.tile([P, S], F32, tag="work")
                    for m in range(MT_LAT):
                        nc.tensor.matmul(pt[:], phi_sb[:, m, j * P:(j + 1) * P],
                                         theta[:, m, :], start=(m == 0),
                                         stop=(m == MT_LAT - 1))
                    nc.scalar.activation(e_sb[:, lk, :], pt[:], AF.Exp,
                                         scale=INV_SQRT_LAT)
                    if lk == 0:
                        nc.vector.tensor_copy(dacc[:], e_sb[:, 0, :])
                    else:
                        nc.vector.tensor_tensor(dacc[:], dacc[:], e_sb[:, lk, :],
                                                ALU.add)

    def emit_attn(i):
        sti = state[i]
        g_sb, e_sb = sti["g"], sti["E"]
        with nc.named_scope(f"attn{i}"):
            psu = []
            for m in range(MT_LAT):
                pu = ps_u.tile([P, S], F32, tag="u")
                for lk in range(MT_L):
                    nc.tensor.matmul(pu[:], g_sb[:, lk, m * P:(m + 1) * P],
                                     e_sb[:, lk, :], start=(lk == 0),
                                     stop=(lk == MT_L - 1))
                psu.append(pu)
        sti["psu"] = psu
        if i + 1 < NSAMP:
            nxt_ap = st_d.ap()[i + 1].rearrange("(o p) s -> p o s", p=P)
            pre = state.setdefault(i + 1, {}).setdefault("st_pre", {})
            for k in range(4):
                t = st_p.tile([P, S], BF, tag="st", name="st_pre")
                nc.sync.dma_start(t[:], nxt_ap[:, k, :])
                pre[k] = t

    def emit_norm(i):
        sti = state[i]
        dacc, psu = sti["dacc"], sti["psu"]
        with nc.named_scope(f"attn{i}"):
            # D = column sums of dacc across partitions, computed on GpSimd so
            # the PE never blocks on the softmax denominator.
            from concourse import bass_isa
            dall = tmp_p.tile([P, S], F32, tag="dall")
            nc.gpsimd.partition_all_reduce(dall[:], dacc[:], P, bass_isa.ReduceOp.add)
            rb = tmp_p.tile([P, S], F32, tag="rb")
            nc.vector.reciprocal(rb[:], dall[:])
            att = att_p.tile([P, MT_LAT, S], F32, tag="att")
            for m in range(MT_LAT):
                nc.vector.tensor_tensor(att[:, m, :], psu[m][:], rb[:], ALU.mult)
                nc.vector.tensor_scalar(att[:, m, :], att[:, m, :],
                                        b_g[:, m:m + 1], None, ALU.add)
        sti["att"] = att

    def emit_ln(i):
        sti = state[i]
        att = sti["att"]
        with nc.named_scope(f"ln{i}"):
            # per-partition stats over the 4*S free elems
            stats = small_p.tile([P, MT_LAT, nc.vector.BN_STATS_DIM], F32, tag="bns")
            for m in range(MT_LAT):
                nc.vector.bn_stats(stats[:, m, :], att[:, m, :])
            mv = small_p.tile([P, nc.vector.BN_AGGR_DIM], F32, tag="bnm")
            nc.vector.bn_aggr(mv[:], stats[:])
            # pack [mean_p, mean_p^2 + var_p] then reduce across partitions
            t2 = small_p.tile([P, 2], F32, tag="t2")
            nc.vector.tensor_copy(t2[:, 0:1], mv[:, 0:1])
            nc.vector.tensor_tensor(t2[:, 1:2], mv[:, 0:1], mv[:, 0:1], ALU.mult)
            nc.vector.tensor_tensor(t2[:, 1:2], t2[:, 1:2], mv[:, 1:2], ALU.add)
            from concourse import bass_isa
            t2r = small_p.tile([P, 2], F32, tag="t2r")
            nc.gpsimd.partition_all_reduce(t2r[:], t2[:], P, bass_isa.ReduceOp.add)
            sg = small_p.tile([P, 4], F32, tag="sg")
            # sg[:,0]=mu, sg[:,1]=E[x^2] (same value on every partition)
            nc.scalar.mul(sg[:, 0:2], t2r[:], 1.0 / P)
            # var = E[x^2] - mu^2 -> sg[:,2]
            nc.vector.tensor_tensor(sg[:, 2:3], sg[:, 0:1], sg[:, 0:1], ALU.mult)
            nc.vector.tensor_tensor(sg[:, 2:3], sg[:, 1:2], sg[:, 2:3], ALU.subtract)
            # rstd = 1/sqrt(var + eps) -> sg[:,3]
            nc.scalar.activation(sg[:, 3:4], sg[:, 2:3], AF.Sqrt, bias=eps_t[:])
            nc.vector.reciprocal(sg[:, 3:4], sg[:, 3:4])
            musd_b = sg
            y_sb = y_p.tile([P, MT_LAT, S], BF, tag="y")
            for m in range(MT_LAT):
                t = tmp_p.tile([P, S], F32, tag="lnt")
                nc.vector.tensor_scalar(t[:], att[:, m, :], musd_b[:, 0:1],
                                        musd_b[:, 3:4], ALU.subtract, ALU.mult)
                nc.vector.tensor_tensor(t[:], t[:], ln_w[:, m, :], ALU.mult)
                nc.vector.tensor_tensor(t[:], t[:], ln_b[:, m, :], ALU.add)
                nc.scalar.activation(y_sb[:, m, :], t[:], AF.Relu)
        sti["y"] = y_sb

    def emit_out(i, act_only=False, dve_only=False):
        sti = state[i]
        y_sb = sti["y"]
        with nc.named_scope(f"out{i}"):
            out_ap = out_d.ap()[i].rearrange("(o p) s -> p o s", p=P)
            for mo in range(MT_C):
                pool = ps_w if mo % 2 == 0 else ps_u
                pt = pool.tile([P, S], F32, tag="work" if mo % 2 == 0 else "u")
                for k in range(MT_LAT):
                    nc.tensor.matmul(pt[:], w_outT[:, k, mo * P:(mo + 1) * P],
                                     y_sb[:, k, :], start=(k == 0),
                                     stop=(k == MT_LAT - 1))
                ot = ost_p.tile([P, S], F32, tag="ost")
                if not dve_only and (act_only or mo % 2 == 0):
                    nc.scalar.activation(ot[:], pt[:], AF.Identity,
                                         bias=b_out[:, mo:mo + 1])
                else:
                    nc.vector.tensor_scalar(ot[:], pt[:], b_out[:, mo:mo + 1],
                                            None, ALU.add)
                nc.sync.dma_start(out_ap[:, mo, :], ot[:])

    # Software-pipelined emission. Sample i's LN chain (Vector-serial) is
    # emitted right after sample i+1's theta so it runs at the front of the
    # Vector queue while the PE does theta + chunk0 (~48us); the final matmuls
    # follow chunk0. Sample 0's final matmuls are held back to the tail where
    # they cover sample 3's LN chain latency.
    for i in range(NSAMP):
        emit_theta(i, extra_dmas=_startup_dmas() if i == 0 else ())
        if i > 0:
            emit_norm(i - 1)
            emit_ln(i - 1)
        emit_chunk(i, 0)
        if i == 0:
            load_late_consts()
        if i > 2:
            emit_out(i - 1)
        for cix in range(1, NCHUNK):
            emit_chunk(i, cix)
        emit_attn(i)
    emit_norm(NSAMP - 1)
    emit_out(0, act_only=True)
    emit_ln(NSAMP - 1)
    emit_out(1, dve_only=True)
    emit_out(NSAMP - 1)
    ctx.close()


_NC_CACHE = None


def _get_nc():
    global _NC_CACHE
    if _NC_CACHE is None:
        _NC_CACHE = build_nc()
    return _NC_CACHE


def kernel(st_feat, lt_feat, w_st, b_st, w_lt, b_lt, w_g, b_g,
           ln_w, ln_b, w_out, b_out):
    n = st_feat.shape[0]
    assert n == N_CORES * NSAMP
    bf16 = ml_dtypes.bfloat16
    st = np.asarray(st_feat, dtype=np.float32).reshape(n, C, S).astype(bf16)
    lt = np.asarray(lt_feat, dtype=np.float32).reshape(n, C, L).astype(bf16)
    w_stT = np.ascontiguousarray(np.asarray(w_st, np.float32).T).astype(bf16)
    w_ltT = np.ascontiguousarray(np.asarray(w_lt, np.float32).T).astype(bf16)
    w_gT = np.ascontiguousarray(np.asarray(w_g, np.float32).T).astype(bf16)
    w_outT = np.ascontiguousarray(np.asarray(w_out, np.float32).T).astype(bf16)
    shared = {
        "w_stT": w_stT, "w_ltT": w_ltT, "w_gT": w_gT, "w_outT": w_outT,
        "b_st": np.asarray(b_st, np.float32), "b_lt": np.asarray(b_lt, np.float32),
        "b_g": np.asarray(b_g, np.float32), "b_out": np.asarray(b_out, np.float32),
        "ln_w": np.ascontiguousarray(np.asarray(ln_w, np.float32)),
        "ln_b": np.ascontiguousarray(np.asarray(ln_b, np.float32)),
    }
    in_maps = []
    for c in range(N_CORES):
        sl = slice(c * NSAMP, (c + 1) * NSAMP)
        in_maps.append({"st": np.ascontiguousarray(st[sl]),
                        "lt": np.ascontiguousarray(lt[sl]), **shared})
    nc = _get_nc()
    res = bass_utils.run_bass_kernel_spmd(nc, in_maps, core_ids=list(range(N_CORES)))
    out = np.concatenate([res.results[c]["out"] for c in range(N_CORES)], axis=0)
    return out.reshape(n, C, S, 1, 1).astype(np.float32)
